# revision 9
# baseline (speedup 1.0000x reference)
"""Trainium2 Bass kernel for nn_MLDecoder (moe_routing).

Data-parallel over batch across 8 NeuronCores (32 batch rows/core, head params
replicated). Activations stay feature-major ("transposed"): C^T = W^T A^T via
matmul(out=C^T, lhsT=W(natural), rhs=A^T). Rows r = b*100+g (b-major). The
batch-independent query path (tgt0, q) is constant-folded on the host. All
matmuls bf16 with fp32 PSUM; LN stats via ones-matmuls; softmax without
max-subtraction (scores are O(1) for this head).
"""
import sys
sys.path.insert(0, "/opt/trn_rl_repo")

import numpy as np
import ml_dtypes

import concourse.bass as bass
from concourse import bacc
import concourse.tile as tile
import concourse.mybir as mybir
from concourse.bass import AP
from concourse.bass_utils import run_bass_kernel_spmd
from concourse.masks import make_identity

F32 = mybir.dt.float32
BF16 = mybir.dt.bfloat16
BF = ml_dtypes.bfloat16
AF = mybir.ActivationFunctionType
ALU = mybir.AluOpType
AX = mybir.AxisListType

B, S, C0 = 256, 49, 2048
D, F = 768, 2048
G, DF = 100, 96
H, HD = 8, 96
EPS = 1e-5
NCORES = 8
BL = B // NCORES          # 32 batch rows per core
R = BL * G                # 3200 rows (b,g) per core
RC = 400                  # row chunk = 4 b
NCHUNK = R // RC
XCH = 4                   # x col chunks (8 b each)
XCOLS = (BL // XCH) * S   # 392
PADS = 64                 # padded spatial stride
MCOLS = BL * PADS         # 2048 padded mem cols


def _bf(a):
    return np.ascontiguousarray(a.astype(BF))


def _ap(base, free_dims):
    """Replace the free dims of a (sliced) AP, keeping its partition dim."""
    return AP(tensor=base.tensor, offset=base.offset,
              ap=[base.ap[0]] + [list(fd) for fd in free_dims])


def build_program():
    nc = bacc.Bacc("TRN2", target_bir_lowering=False, debug=False,
                   num_devices=NCORES)
    d = {}

    def din(name, shape, dt):
        d[name] = nc.dram_tensor(name, list(shape), dt, kind="ExternalInput").ap()

    din("xT", (XCH, 128, 16 * XCOLS), BF16)
    din("wemb", (128, 16 * 768), BF16)
    din("be", (128, 6), F32)
    din("wk", (128, 6 * 768), BF16)
    din("wv", (128, 6 * 768), BF16)
    din("wao", (96, 8 * 768), BF16)
    din("bao", (128, 6), F32)
    din("w1", (128, 6 * 2048), BF16)
    din("b1", (128, 16), F32)
    din("w2", (128, 16 * 768), BF16)
    din("b2", (128, 6), F32)
    din("qT", (96, 8 * 100), BF16)
    din("qbk", (100, 8), F32)
    din("tgt0", (128, 6 * 100), BF16)
    din("ln2g", (128, 6), F32)
    din("ln2b", (128, 6), F32)
    din("ln3g", (128, 6), F32)
    din("ln3b", (128, 6), F32)
    din("dup", (100, 128, 6 * 96), BF16)
    din("dupb", (1, G * 96), BF16)
    out_d = nc.dram_tensor("logitsT", [96, G * BL], F32,
                           kind="ExternalOutput").ap()

    with tile.TileContext(nc) as tc:
        build_kernel(tc, d, out_d)
    nc.compile()
    return nc


def build_kernel(tc, d, out_d):
    nc = tc.nc

    def pool(name, bufs=1, space="SBUF"):
        return tc.tile_pool(name=name, bufs=bufs, space=space)

    with pool("resident") as res, pool("h3pool") as h3p, pool("oTpool") as oTp:
        ident = res.tile([128, 128], BF16)
        make_identity(nc, ident[:])
        ones_col = res.tile([128, 1], BF16)
        nc.vector.memset(ones_col[:], 1.0)
        ones_row = res.tile([1, 128], BF16)
        nc.vector.memset(ones_row[:], 1.0)
        ones32 = res.tile([1, BL], BF16)
        nc.vector.memset(ones32[:], 1.0)
        eps_t = res.tile([1, 1], F32)
        nc.vector.memset(eps_t[:], EPS)

        small = {}
        for name in ["be", "bao", "b1", "b2", "qT", "qbk", "tgt0",
                     "ln2g", "ln2b", "ln3g", "ln3b"]:
            t = res.tile(list(d[name].shape), d[name].dtype, tag=name)
            nc.sync.dma_start(out=t, in_=d[name])
            small[name] = t

        h3T = h3p.tile([128, 6 * R], BF16)
        oT = oTp.tile([96, 8 * R], BF16)

        with pool("memTpool") as memp:
            memT = memp.tile([128, 6 * MCOLS], BF16)

            # ---- P0: mem^T = relu(We^T x^T + be), written b-padded ----
            with pool("p0w") as p0w, pool("p0x", bufs=2) as p0x, \
                 pool("p0ps", bufs=3, space="PSUM") as p0ps:
                wemb = p0w.tile([128, 16 * 768], BF16)
                nc.sync.dma_start(out=wemb, in_=d["wemb"])
                for c in range(XCH):
                    xt = p0x.tile([128, 16 * XCOLS], BF16)
                    nc.sync.dma_start(out=xt, in_=d["xT"][c])
                    for m in range(6):
                        ps = p0ps.tile([128, XCOLS], F32)
                        for k in range(16):
                            nc.tensor.matmul(
                                ps[:],
                                wemb[:, k * 768 + m * 128:k * 768 + m * 128 + 128],
                                xt[:, k * XCOLS:(k + 1) * XCOLS],
                                start=(k == 0), stop=(k == 15))
                        dst = _ap(memT[:, m * MCOLS + c * 8 * PADS:],
                                  [[PADS, 8], [1, S]])
                        src = _ap(ps[:], [[S, 8], [1, S]])
                        nc.scalar.activation(out=dst, in_=src, func=AF.Relu,
                                             bias=small["be"][:, m:m + 1],
                                             scale=1.0)

            # ---- P1: K^T (head-major, b-padded) and V (rows padded) ----
            with pool("kvpool") as kvp:
                KT = kvp.tile([96, 8 * MCOLS], BF16)
                Vp = kvp.tile([128, 16 * 768], BF16)
                with pool("p1w") as p1w, \
                     pool("p1ps", bufs=3, space="PSUM") as p1ps:
                    wk = p1w.tile([128, 6 * 768], BF16)
                    nc.sync.dma_start(out=wk, in_=d["wk"])
                    wv = p1w.tile([128, 6 * 768], BF16)
                    nc.sync.dma_start(out=wv, in_=d["wv"])
                    for h in range(H):
                        for c in range(XCH):
                            ps = p1ps.tile([96, XCOLS], F32)
                            for k in range(6):
                                rhs = _ap(memT[:, k * MCOLS + c * 8 * PADS:],
                                          [[PADS, 8], [1, S]])
                                nc.tensor.matmul(
                                    ps[:],
                                    wk[:, k * 768 + h * 96:k * 768 + h * 96 + 96],
                                    rhs, start=(k == 0), stop=(k == 5))
                            dst = _ap(KT[:, h * MCOLS + c * 8 * PADS:],
                                      [[PADS, 8], [1, S]])
                            nc.vector.tensor_copy(
                                out=dst, in_=_ap(ps[:], [[S, 8], [1, S]]))
                    for t in range(16):
                        ps = p1ps.tile([128, 768], F32)
                        for sub in range(2):
                            n0, n1 = sub * 512, min(768, (sub + 1) * 512)
                            for k in range(6):
                                nc.tensor.matmul(
                                    ps[:, n0:n1],
                                    memT[:, k * MCOLS + t * 128:
                                         k * MCOLS + t * 128 + 128],
                                    wv[:, k * 768 + n0:k * 768 + n1],
                                    start=(k == 0), stop=(k == 5))
                        nc.vector.tensor_copy(out=Vp[:, t * 768:(t + 1) * 768],
                                              in_=ps[:])

                # ---- P2: attention ----
                with pool("p2a", bufs=2) as p2a, pool("p2s", bufs=3) as p2s, \
                     pool("p2ps", bufs=2, space="PSUM") as psc, \
                     pool("p2pt", bufs=2, space="PSUM") as pst, \
                     pool("p2po", bufs=2, space="PSUM") as pso:
                    for bg in range(4):
                        attnT = p2a.tile([128, 8 * 400], BF16)
                        for h in range(H):
                            ps = psc.tile([100, 8 * S], F32)
                            rhs = _ap(KT[:, h * MCOLS + bg * 8 * PADS:],
                                      [[PADS, 8], [1, S]])
                            nc.tensor.matmul(ps[:],
                                             small["qT"][:, h * 100:(h + 1) * 100],
                                             rhs, start=True, stop=True)
                            # exp into 64-padded slots (pads hold garbage,
                            # excluded by every later access pattern)
                            att = p2s.tile([100, 8 * PADS], BF16)
                            nc.scalar.activation(out=_ap(att[:], [[PADS, 8], [1, S]]),
                                                 in_=ps[:],
                                                 func=AF.Exp,
                                                 bias=small["qbk"][:, h:h + 1],
                                                 scale=1.0)
                            sums = p2s.tile([100, 8], F32)
                            nc.vector.reduce_sum(out=sums[:],
                                                 in_=_ap(att[:], [[PADS, 8], [1, S]]),
                                                 axis=AX.X)
                            inv = p2s.tile([100, 8], F32)
                            nc.vector.reciprocal(out=inv[:], in_=sums[:])
                            attn = p2s.tile([100, 8 * PADS], BF16)
                            nc.vector.tensor_tensor(
                                out=_ap(attn[:], [[PADS, 8], [1, S]]),
                                in0=_ap(att[:], [[PADS, 8], [1, S]]),
                                in1=_ap(inv[:], [[1, 8], [0, S]]),
                                op=ALU.mult)
                            for pr in range(4):
                                pt = pst.tile([128, 100], BF16)
                                nc.tensor.transpose(
                                    pt[:], attn[:, pr * 128:(pr + 1) * 128],
                                    ident[0:100, 0:100])
                                nc.vector.tensor_copy(
                                    out=attnT[:, h * 400 + pr * 100:
                                              h * 400 + pr * 100 + 100],
                                    in_=pt[:])
                        for lb in range(8):
                            b = bg * 8 + lb
                            po = pso.tile([96, 1024], F32)
                            for h in range(H):
                                vsl = Vp[(lb % 2) * 64:(lb % 2) * 64 + S,
                                         (b // 2) * 768 + h * 96:
                                         (b // 2) * 768 + h * 96 + 96]
                                nc.tensor.matmul(
                                    po[:, h * 128:h * 128 + 100], vsl,
                                    attnT[(lb % 2) * 64:(lb % 2) * 64 + S,
                                          h * 400 + (lb // 2) * 100:
                                          h * 400 + (lb // 2) * 100 + 100],
                                    start=True, stop=True)
                            dst = _ap(oT[:, b * 100:], [[R, 8], [1, 100]])
                            nc.vector.tensor_copy(
                                out=dst, in_=_ap(po[:], [[128, 8], [1, 100]]))

        # ---- P3: attn_out + LN2 + FFN + LN3 -> h3T ----
        with pool("p3w") as p3w, pool("p3t") as p3t, \
             pool("p3f") as p3f, pool("p3s", bufs=2) as p3s, \
             pool("p3ps", bufs=2, space="PSUM") as p3ps, \
             pool("p3st", space="PSUM") as p3st, \
             pool("p3ab", space="PSUM") as p3ab:
            wao = p3w.tile([96, 8 * 768], BF16)
            nc.sync.dma_start(out=wao, in_=d["wao"])
            w1 = p3w.tile([128, 6 * 2048], BF16)
            nc.sync.dma_start(out=w1, in_=d["w1"])
            w2 = p3w.tile([128, 16 * 768], BF16)
            nc.sync.dma_start(out=w2, in_=d["w2"])

            def layer_norm_T(xin, gname, bname, yout):
                sq = p3f.tile([128, 6 * RC], BF16)
                nc.scalar.square(out=sq[:], in_=xin[:])
                s1 = p3st.tile([1, RC], F32)
                s2 = p3st.tile([1, RC], F32)
                for k in range(6):
                    nc.tensor.matmul(s1[:], ones_col[:],
                                     xin[:, k * RC:(k + 1) * RC],
                                     start=(k == 0), stop=(k == 5))
                for k in range(6):
                    nc.tensor.matmul(s2[:], ones_col[:],
                                     sq[:, k * RC:(k + 1) * RC],
                                     start=(k == 0), stop=(k == 5))
                mean = p3f.tile([1, RC], F32)
                nc.vector.tensor_scalar_mul(out=mean[:], in0=s1[:],
                                            scalar1=1.0 / D)
                var = p3f.tile([1, RC], F32)
                nc.vector.tensor_scalar_mul(out=var[:], in0=s2[:],
                                            scalar1=1.0 / D)
                msq = p3f.tile([1, RC], F32)
                nc.vector.tensor_tensor(out=msq[:], in0=mean[:], in1=mean[:],
                                        op=ALU.mult)
                nc.vector.tensor_tensor(out=var[:], in0=var[:], in1=msq[:],
                                        op=ALU.subtract)
                sd = p3f.tile([1, RC], F32)
                nc.scalar.activation(out=sd[:], in_=var[:], func=AF.Sqrt,
                                     bias=eps_t[:], scale=1.0)
                rstd = p3f.tile([1, RC], F32)
                nc.vector.reciprocal(out=rstd[:], in_=sd[:])
                nmr = p3f.tile([1, RC], F32)
                nc.vector.tensor_tensor(out=nmr[:], in0=mean[:], in1=rstd[:],
                                        op=ALU.mult)
                nc.vector.tensor_scalar_mul(out=nmr[:], in0=nmr[:], scalar1=-1.0)
                rstd_b = p3f.tile([1, RC], BF16)
                nc.vector.tensor_copy(out=rstd_b[:], in_=rstd[:])
                nmr_b = p3f.tile([1, RC], BF16)
                nc.vector.tensor_copy(out=nmr_b[:], in_=nmr[:])
                pa = p3ab.tile([128, RC], F32)
                nc.tensor.matmul(pa[:], ones_row[:], rstd_b[:],
                                 start=True, stop=True)
                pb = p3ab.tile([128, RC], F32)
                nc.tensor.matmul(pb[:], ones_row[:], nmr_b[:],
                                 start=True, stop=True)
                gv, bv = small[gname], small[bname]
                for k in range(6):
                    u = p3s.tile([128, RC], F32)
                    nc.vector.tensor_tensor(out=u[:],
                                            in0=xin[:, k * RC:(k + 1) * RC],
                                            in1=pa[:], op=ALU.mult)
                    nc.vector.tensor_tensor(out=u[:], in0=u[:], in1=pb[:],
                                            op=ALU.add)
                    nc.vector.tensor_scalar(out=yout(k), in0=u[:],
                                            scalar1=gv[:, k:k + 1],
                                            scalar2=bv[:, k:k + 1],
                                            op0=ALU.mult, op1=ALU.add)

            for c in range(NCHUNK):
                t2 = p3t.tile([128, 6 * RC], BF16)
                for m in range(6):
                    ps = p3ps.tile([128, RC], F32)
                    for kh in range(H):
                        nc.tensor.matmul(
                            ps[:],
                            wao[:, kh * 768 + m * 128:kh * 768 + m * 128 + 128],
                            oT[:, kh * R + c * RC:kh * R + (c + 1) * RC],
                            start=(kh == 0), stop=(kh == 7))
                    ta = p3s.tile([128, RC], BF16)
                    nc.scalar.activation(out=ta[:], in_=ps[:], func=AF.Identity,
                                         bias=small["bao"][:, m:m + 1], scale=1.0)
                    tg = small["tgt0"][:, m * 100:(m + 1) * 100]
                    nc.vector.tensor_tensor(out=t2[:, m * RC:(m + 1) * RC],
                                            in0=ta[:],
                                            in1=_ap(tg, [[0, 4], [1, 100]]),
                                            op=ALU.add)
                y2 = p3t.tile([128, 6 * RC], BF16)
                layer_norm_T(t2, "ln2g", "ln2b",
                             lambda k: y2[:, k * RC:(k + 1) * RC])
                ff1 = p3f.tile([128, 16 * RC], BF16)
                for mf in range(16):
                    ps = p3ps.tile([128, RC], F32)
                    for k in range(6):
                        nc.tensor.matmul(
                            ps[:],
                            w1[:, k * 2048 + mf * 128:k * 2048 + mf * 128 + 128],
                            y2[:, k * RC:(k + 1) * RC],
                            start=(k == 0), stop=(k == 5))
                    nc.scalar.activation(out=ff1[:, mf * RC:(mf + 1) * RC],
                                         in_=ps[:], func=AF.Relu,
                                         bias=small["b1"][:, mf:mf + 1],
                                         scale=1.0)
                t3 = p3t.tile([128, 6 * RC], BF16)
                for m in range(6):
                    ps = p3ps.tile([128, RC], F32)
                    for k in range(16):
                        nc.tensor.matmul(
                            ps[:],
                            w2[:, k * 768 + m * 128:k * 768 + m * 128 + 128],
                            ff1[:, k * RC:(k + 1) * RC],
                            start=(k == 0), stop=(k == 15))
                    tb = p3s.tile([128, RC], BF16)
                    nc.scalar.activation(out=tb[:], in_=ps[:], func=AF.Identity,
                                         bias=small["b2"][:, m:m + 1], scale=1.0)
                    nc.vector.tensor_tensor(out=t3[:, m * RC:(m + 1) * RC],
                                            in0=tb[:],
                                            in1=y2[:, m * RC:(m + 1) * RC],
                                            op=ALU.add)
                layer_norm_T(t3, "ln3g", "ln3b",
                             lambda k: h3T[:, k * R + c * RC:k * R + (c + 1) * RC])

        # ---- P4: GroupFC -> logitsT ----
        with pool("p4d", bufs=3) as p4d, pool("p4o") as p4o, \
             pool("p4ps", bufs=2, space="PSUM") as p4ps:
            logitsT = p4o.tile([96, G * BL], F32)
            dupb = p4o.tile(list(d["dupb"].shape), BF16)
            nc.sync.dma_start(out=dupb, in_=d["dupb"])
            for g0 in range(0, G, 16):
                ng = min(16, G - g0)
                ps = p4ps.tile([96, 16 * BL], F32)
                for gi in range(ng):
                    g = g0 + gi
                    dup = p4d.tile([128, 6 * 96], BF16)
                    nc.sync.dma_start(out=dup, in_=d["dup"][g])
                    nc.tensor.matmul(ps[:, gi * BL:(gi + 1) * BL],
                                     dupb[:, g * 96:(g + 1) * 96],
                                     ones32[:], start=True, stop=False)
                    for k in range(6):
                        hsl = _ap(h3T[:, k * R + g:], [[100, BL]])
                        nc.tensor.matmul(ps[:, gi * BL:(gi + 1) * BL],
                                         dup[:, k * 96:(k + 1) * 96],
                                         hsl, start=False, stop=(k == 5))
                nc.vector.tensor_copy(out=logitsT[:, g0 * BL:(g0 + ng) * BL],
                                      in_=ps[:, 0:ng * BL])
            nc.sync.dma_start(out=out_d, in_=logitsT[:])


_CACHE = {}


def kernel(**inputs):
    f32 = lambda k: np.asarray(inputs[k], np.float32)
    x = f32("x")
    w_qkv, b_qkv = f32("w_qkv"), f32("b_qkv")
    w_attn_out, b_attn_out = f32("w_attn_out"), f32("b_attn_out")

    # host constant folding for the batch-independent query path
    t = 2.0 * f32("query_embed")
    mu = t.mean(-1, keepdims=True)
    va = ((t - mu) ** 2).mean(-1, keepdims=True)
    tgt0 = (t - mu) / np.sqrt(va + EPS) * f32("ln1_g") + f32("ln1_b")
    q = (tgt0 @ w_qkv[:, :D] + b_qkv[:D]) / np.sqrt(float(HD))
    bk = b_qkv[D:2 * D]
    qbk = np.stack([q[:, h * HD:(h + 1) * HD] @ bk[h * HD:(h + 1) * HD]
                    for h in range(H)], axis=1)
    bv = b_qkv[2 * D:]
    bao_eff = b_attn_out + bv @ w_attn_out   # softmax rows sum to 1

    col6 = lambda a: np.ascontiguousarray(a.reshape(6, 128).T)
    feed = {
        "wemb": _bf(f32("w_embed").reshape(16, 128, 768).transpose(1, 0, 2)
                    .reshape(128, -1)),
        "be": col6(f32("b_embed")),
        "wk": _bf(w_qkv[:, D:2 * D].reshape(6, 128, 768).transpose(1, 0, 2)
                  .reshape(128, -1)),
        "wv": _bf(w_qkv[:, 2 * D:].reshape(6, 128, 768).transpose(1, 0, 2)
                  .reshape(128, -1)),
        "wao": _bf(w_attn_out.reshape(8, 96, 768).transpose(1, 0, 2)
                   .reshape(96, -1)),
        "bao": col6(bao_eff),
        "w1": _bf(f32("w1").reshape(6, 128, 2048).transpose(1, 0, 2)
                  .reshape(128, -1)),
        "b1": np.ascontiguousarray(f32("b1").reshape(16, 128).T),
        "w2": _bf(f32("w2").reshape(16, 128, 768).transpose(1, 0, 2)
                  .reshape(128, -1)),
        "b2": col6(f32("b2")),
        "qT": _bf(q.T.reshape(8, 96, 100).transpose(1, 0, 2).reshape(96, -1)),
        "qbk": np.ascontiguousarray(qbk.astype(np.float32)),
        "tgt0": _bf(tgt0.T.reshape(6, 128, 100).transpose(1, 0, 2)
                    .reshape(128, -1)),
        "ln2g": col6(f32("ln2_g")), "ln2b": col6(f32("ln2_b")),
        "ln3g": col6(f32("ln3_g")), "ln3b": col6(f32("ln3_b")),
        "dup": _bf(f32("dup_pool").reshape(G, 6, 128, 96).transpose(0, 2, 1, 3)
                   .reshape(G, 128, 6 * 96)),
        "dupb": _bf(f32("dup_bias").reshape(1, -1)),
    }

    if "nc" not in _CACHE:
        _CACHE["nc"] = build_program()
    nc = _CACHE["nc"]

    # xr[core] axes: [c, col, k, p]; device wants [c, p, k, col]
    xr = x.reshape(NCORES, XCH, XCOLS, 16, 128)
    in_maps = []
    for core in range(NCORES):
        xT = xr[core].transpose(0, 3, 2, 1).reshape(XCH, 128, 16 * XCOLS)
        in_maps.append({**feed, "xT": _bf(xT)})

    _CACHE["in_maps"] = in_maps
    res = run_bass_kernel_spmd(nc, in_maps, list(range(NCORES)))
    outs = []
    for core in range(NCORES):
        lt = np.asarray(res.results[core]["logitsT"], np.float32)
        outs.append(lt.reshape(96, G, BL).transpose(2, 1, 0).reshape(BL, G * DF))
    return np.concatenate(outs, axis=0).astype(np.float32)


# revision 10
# speedup vs baseline: 1.0547x; 1.0547x over previous
"""Trainium2 Bass kernel for nn_MLDecoder (moe_routing).

Data-parallel over batch across 8 NeuronCores (32 batch rows/core, head params
replicated). Activations stay feature-major ("transposed"): C^T = W^T A^T via
matmul(out=C^T, lhsT=W(natural), rhs=A^T). Rows r = b*100+g (b-major). The
batch-independent query path (tgt0, q) is constant-folded on the host. All
matmuls bf16 with fp32 PSUM; LN stats via ones-matmuls; softmax without
max-subtraction (scores are O(1) for this head).
"""
import sys
sys.path.insert(0, "/opt/trn_rl_repo")

import numpy as np
import ml_dtypes

import concourse.bass as bass
from concourse import bacc
import concourse.tile as tile
import concourse.mybir as mybir
from concourse.bass import AP
from concourse.bass_utils import run_bass_kernel_spmd
from concourse.masks import make_identity

F32 = mybir.dt.float32
BF16 = mybir.dt.bfloat16
BF = ml_dtypes.bfloat16
AF = mybir.ActivationFunctionType
ALU = mybir.AluOpType
AX = mybir.AxisListType

B, S, C0 = 256, 49, 2048
D, F = 768, 2048
G, DF = 100, 96
H, HD = 8, 96
EPS = 1e-5
NCORES = 8
BL = B // NCORES          # 32 batch rows per core
R = BL * G                # 3200 rows (b,g) per core
RC = 400                  # row chunk = 4 b
NCHUNK = R // RC
XCH = 4                   # x col chunks (8 b each)
XCOLS = (BL // XCH) * S   # 392
PADS = 64                 # padded spatial stride
MCOLS = BL * PADS         # 2048 padded mem cols


def _bf(a):
    return np.ascontiguousarray(a.astype(BF))


def _ap(base, free_dims):
    """Replace the free dims of a (sliced) AP, keeping its partition dim."""
    return AP(tensor=base.tensor, offset=base.offset,
              ap=[base.ap[0]] + [list(fd) for fd in free_dims])


def build_program():
    nc = bacc.Bacc("TRN2", target_bir_lowering=False, debug=False,
                   num_devices=NCORES)
    d = {}

    def din(name, shape, dt):
        d[name] = nc.dram_tensor(name, list(shape), dt, kind="ExternalInput").ap()

    din("xT", (XCH, 128, 16 * XCOLS), BF16)
    din("wemb", (128, 16 * 768), BF16)
    din("be", (128, 6), F32)
    din("wk", (128, 6 * 768), BF16)
    din("wv", (128, 6 * 768), BF16)
    din("wao", (96, 8 * 768), BF16)
    din("bao", (128, 6), F32)
    din("w1", (128, 6 * 2048), BF16)
    din("b1", (128, 16), F32)
    din("w2", (128, 16 * 768), BF16)
    din("b2", (128, 6), F32)
    din("qT", (96, 8 * 100), BF16)
    din("qbk", (100, 8), F32)
    din("tgt0", (128, 6 * 100), BF16)
    din("ln2g", (128, 6), F32)
    din("ln2b", (128, 6), F32)
    din("ln3g", (128, 6), F32)
    din("ln3b", (128, 6), F32)
    din("dup", (100, 128, 6 * 96), BF16)
    din("dupb", (1, G * 96), BF16)
    out_d = nc.dram_tensor("logitsT", [96, G * BL], F32,
                           kind="ExternalOutput").ap()

    with tile.TileContext(nc) as tc:
        build_kernel(tc, d, out_d)
    nc.compile()
    return nc


def build_kernel(tc, d, out_d):
    nc = tc.nc

    def pool(name, bufs=1, space="SBUF"):
        return tc.tile_pool(name=name, bufs=bufs, space=space)

    with pool("resident") as res, pool("h3pool") as h3p, pool("oTpool") as oTp:
        ident = res.tile([128, 128], BF16)
        make_identity(nc, ident[:])
        ones_col = res.tile([128, 1], BF16)
        nc.vector.memset(ones_col[:], 1.0)
        ones_row = res.tile([1, 128], BF16)
        nc.vector.memset(ones_row[:], 1.0)
        ones32 = res.tile([1, BL], BF16)
        nc.vector.memset(ones32[:], 1.0)
        eps_t = res.tile([1, 1], F32)
        nc.vector.memset(eps_t[:], EPS)

        small = {}
        for name in ["be", "bao", "b1", "b2", "qT", "qbk", "tgt0",
                     "ln2g", "ln2b", "ln3g", "ln3b"]:
            t = res.tile(list(d[name].shape), d[name].dtype, tag=name)
            nc.sync.dma_start(out=t, in_=d[name])
            small[name] = t

        h3T = h3p.tile([128, 6 * R], BF16)
        oT = oTp.tile([96, 8 * R], BF16)

        with pool("memTpool") as memp:
            memT = memp.tile([128, 6 * MCOLS], BF16)

            # ---- P0: mem^T = relu(We^T x^T + be), written b-padded ----
            with pool("p0w") as p0w, pool("p0x", bufs=2) as p0x, \
                 pool("p0ps", bufs=3, space="PSUM") as p0ps:
                wemb = p0w.tile([128, 16 * 768], BF16)
                nc.sync.dma_start(out=wemb, in_=d["wemb"])
                for c in range(XCH):
                    xt = p0x.tile([128, 16 * XCOLS], BF16)
                    nc.sync.dma_start(out=xt, in_=d["xT"][c])
                    for m in range(6):
                        ps = p0ps.tile([128, XCOLS], F32)
                        for k in range(16):
                            nc.tensor.matmul(
                                ps[:],
                                wemb[:, k * 768 + m * 128:k * 768 + m * 128 + 128],
                                xt[:, k * XCOLS:(k + 1) * XCOLS],
                                start=(k == 0), stop=(k == 15))
                        dst = _ap(memT[:, m * MCOLS + c * 8 * PADS:],
                                  [[PADS, 8], [1, S]])
                        src = _ap(ps[:], [[S, 8], [1, S]])
                        nc.scalar.activation(out=dst, in_=src, func=AF.Relu,
                                             bias=small["be"][:, m:m + 1],
                                             scale=1.0)

            # ---- P1: K^T (head-major, b-padded) and V (rows padded) ----
            with pool("kvpool") as kvp:
                KT = kvp.tile([96, 8 * MCOLS], BF16)
                Vp = kvp.tile([128, 16 * 768], BF16)
                with pool("p1w") as p1w, \
                     pool("p1ps", bufs=3, space="PSUM") as p1ps:
                    wk = p1w.tile([128, 6 * 768], BF16)
                    nc.sync.dma_start(out=wk, in_=d["wk"])
                    wv = p1w.tile([128, 6 * 768], BF16)
                    nc.sync.dma_start(out=wv, in_=d["wv"])
                    for h in range(H):
                        for c in range(XCH):
                            ps = p1ps.tile([96, XCOLS], F32)
                            for k in range(6):
                                rhs = _ap(memT[:, k * MCOLS + c * 8 * PADS:],
                                          [[PADS, 8], [1, S]])
                                nc.tensor.matmul(
                                    ps[:],
                                    wk[:, k * 768 + h * 96:k * 768 + h * 96 + 96],
                                    rhs, start=(k == 0), stop=(k == 5))
                            dst = _ap(KT[:, h * MCOLS + c * 8 * PADS:],
                                      [[PADS, 8], [1, S]])
                            nc.vector.tensor_copy(
                                out=dst, in_=_ap(ps[:], [[S, 8], [1, S]]))
                    for t in range(16):
                        ps = p1ps.tile([128, 768], F32)
                        for sub in range(2):
                            n0, n1 = sub * 512, min(768, (sub + 1) * 512)
                            for k in range(6):
                                nc.tensor.matmul(
                                    ps[:, n0:n1],
                                    memT[:, k * MCOLS + t * 128:
                                         k * MCOLS + t * 128 + 128],
                                    wv[:, k * 768 + n0:k * 768 + n1],
                                    start=(k == 0), stop=(k == 5))
                        nc.vector.tensor_copy(out=Vp[:, t * 768:(t + 1) * 768],
                                              in_=ps[:])

                # ---- P2: attention ----
                with pool("p2a", bufs=2) as p2a, pool("p2s", bufs=3) as p2s, \
                     pool("p2ps", bufs=2, space="PSUM") as psc, \
                     pool("p2pt", bufs=2, space="PSUM") as pst, \
                     pool("p2po", bufs=2, space="PSUM") as pso:
                    for bg in range(4):
                        attnT = p2a.tile([128, 8 * 400], BF16)
                        for h in range(H):
                            ps = psc.tile([100, 8 * S], F32)
                            rhs = _ap(KT[:, h * MCOLS + bg * 8 * PADS:],
                                      [[PADS, 8], [1, S]])
                            nc.tensor.matmul(ps[:],
                                             small["qT"][:, h * 100:(h + 1) * 100],
                                             rhs, start=True, stop=True)
                            # exp into 64-padded slots (pads hold garbage,
                            # excluded by every later access pattern)
                            att = p2s.tile([100, 8 * PADS], BF16)
                            nc.scalar.activation(out=_ap(att[:], [[PADS, 8], [1, S]]),
                                                 in_=ps[:],
                                                 func=AF.Exp,
                                                 bias=small["qbk"][:, h:h + 1],
                                                 scale=1.0)
                            sums = p2s.tile([100, 8], F32)
                            nc.vector.reduce_sum(out=sums[:],
                                                 in_=_ap(att[:], [[PADS, 8], [1, S]]),
                                                 axis=AX.X)
                            inv = p2s.tile([100, 8], F32)
                            nc.vector.reciprocal(out=inv[:], in_=sums[:])
                            attn = p2s.tile([100, 8 * PADS], BF16)
                            nc.vector.tensor_tensor(
                                out=_ap(attn[:], [[PADS, 8], [1, S]]),
                                in0=_ap(att[:], [[PADS, 8], [1, S]]),
                                in1=_ap(inv[:], [[1, 8], [0, S]]),
                                op=ALU.mult)
                            for pr in range(4):
                                pt = pst.tile([128, 100], BF16)
                                nc.tensor.transpose(
                                    pt[:], attn[:, pr * 128:(pr + 1) * 128],
                                    ident[0:100, 0:100])
                                nc.vector.tensor_copy(
                                    out=attnT[:, h * 400 + pr * 100:
                                              h * 400 + pr * 100 + 100],
                                    in_=pt[:])
                        for lb in range(8):
                            b = bg * 8 + lb
                            po = pso.tile([96, 1024], F32)
                            for h in range(H):
                                vsl = Vp[(lb % 2) * 64:(lb % 2) * 64 + S,
                                         (b // 2) * 768 + h * 96:
                                         (b // 2) * 768 + h * 96 + 96]
                                nc.tensor.matmul(
                                    po[:, h * 128:h * 128 + 100], vsl,
                                    attnT[(lb % 2) * 64:(lb % 2) * 64 + S,
                                          h * 400 + (lb // 2) * 100:
                                          h * 400 + (lb // 2) * 100 + 100],
                                    start=True, stop=True)
                            dst = _ap(oT[:, b * 100:], [[R, 8], [1, 100]])
                            nc.vector.tensor_copy(
                                out=dst, in_=_ap(po[:], [[128, 8], [1, 100]]))

        # ---- P3: attn_out + LN2 + FFN + LN3 -> h3T ----
        with pool("p3w") as p3w, pool("p3t") as p3t, \
             pool("p3f") as p3f, pool("p3s", bufs=2) as p3s, \
             pool("p3ps", bufs=4, space="PSUM") as p3ps, \
             pool("p3st", space="PSUM") as p3st, \
             pool("p3ab", space="PSUM") as p3ab:
            wao = p3w.tile([96, 8 * 768], BF16)
            nc.sync.dma_start(out=wao, in_=d["wao"])
            w1 = p3w.tile([128, 6 * 2048], BF16)
            nc.sync.dma_start(out=w1, in_=d["w1"])
            w2 = p3w.tile([128, 16 * 768], BF16)
            nc.sync.dma_start(out=w2, in_=d["w2"])

            def layer_norm_T(xin, gname, bname, yout):
                sq = p3f.tile([128, 6 * RC], BF16)
                nc.scalar.square(out=sq[:], in_=xin[:])
                s1 = p3st.tile([1, RC], F32)
                s2 = p3st.tile([1, RC], F32)
                for k in range(6):
                    nc.tensor.matmul(s1[:], ones_col[:],
                                     xin[:, k * RC:(k + 1) * RC],
                                     start=(k == 0), stop=(k == 5))
                for k in range(6):
                    nc.tensor.matmul(s2[:], ones_col[:],
                                     sq[:, k * RC:(k + 1) * RC],
                                     start=(k == 0), stop=(k == 5))
                mean = p3f.tile([1, RC], F32)
                nc.vector.tensor_scalar_mul(out=mean[:], in0=s1[:],
                                            scalar1=1.0 / D)
                var = p3f.tile([1, RC], F32)
                nc.vector.tensor_scalar_mul(out=var[:], in0=s2[:],
                                            scalar1=1.0 / D)
                msq = p3f.tile([1, RC], F32)
                nc.vector.tensor_tensor(out=msq[:], in0=mean[:], in1=mean[:],
                                        op=ALU.mult)
                nc.vector.tensor_tensor(out=var[:], in0=var[:], in1=msq[:],
                                        op=ALU.subtract)
                sd = p3f.tile([1, RC], F32)
                nc.scalar.activation(out=sd[:], in_=var[:], func=AF.Sqrt,
                                     bias=eps_t[:], scale=1.0)
                rstd = p3f.tile([1, RC], F32)
                nc.vector.reciprocal(out=rstd[:], in_=sd[:])
                nmr = p3f.tile([1, RC], F32)
                nc.vector.tensor_tensor(out=nmr[:], in0=mean[:], in1=rstd[:],
                                        op=ALU.mult)
                nc.vector.tensor_scalar_mul(out=nmr[:], in0=nmr[:], scalar1=-1.0)
                rstd_b = p3f.tile([1, RC], BF16)
                nc.vector.tensor_copy(out=rstd_b[:], in_=rstd[:])
                nmr_b = p3f.tile([1, RC], BF16)
                nc.vector.tensor_copy(out=nmr_b[:], in_=nmr[:])
                pa = p3ab.tile([128, RC], F32)
                nc.tensor.matmul(pa[:], ones_row[:], rstd_b[:],
                                 start=True, stop=True)
                pb = p3ab.tile([128, RC], F32)
                nc.tensor.matmul(pb[:], ones_row[:], nmr_b[:],
                                 start=True, stop=True)
                gv, bv = small[gname], small[bname]
                for k in range(6):
                    u = p3s.tile([128, RC], F32)
                    nc.vector.tensor_tensor(out=u[:],
                                            in0=xin[:, k * RC:(k + 1) * RC],
                                            in1=pa[:], op=ALU.mult)
                    nc.vector.tensor_tensor(out=u[:], in0=u[:], in1=pb[:],
                                            op=ALU.add)
                    nc.vector.tensor_scalar(out=yout(k), in0=u[:],
                                            scalar1=gv[:, k:k + 1],
                                            scalar2=bv[:, k:k + 1],
                                            op0=ALU.mult, op1=ALU.add)

            for c in range(NCHUNK):
                t2 = p3t.tile([128, 6 * RC], BF16)
                for m in range(6):
                    ps = p3ps.tile([128, RC], F32)
                    for kh in range(H):
                        nc.tensor.matmul(
                            ps[:],
                            wao[:, kh * 768 + m * 128:kh * 768 + m * 128 + 128],
                            oT[:, kh * R + c * RC:kh * R + (c + 1) * RC],
                            start=(kh == 0), stop=(kh == 7))
                    ta = p3s.tile([128, RC], BF16)
                    nc.scalar.activation(out=ta[:], in_=ps[:], func=AF.Identity,
                                         bias=small["bao"][:, m:m + 1], scale=1.0)
                    tg = small["tgt0"][:, m * 100:(m + 1) * 100]
                    nc.vector.tensor_tensor(out=t2[:, m * RC:(m + 1) * RC],
                                            in0=ta[:],
                                            in1=_ap(tg, [[0, 4], [1, 100]]),
                                            op=ALU.add)
                y2 = p3t.tile([128, 6 * RC], BF16)
                layer_norm_T(t2, "ln2g", "ln2b",
                             lambda k: y2[:, k * RC:(k + 1) * RC])
                ff1 = p3f.tile([128, 16 * RC], BF16)
                for mf in range(16):
                    ps = p3ps.tile([128, RC], F32)
                    for k in range(6):
                        nc.tensor.matmul(
                            ps[:],
                            w1[:, k * 2048 + mf * 128:k * 2048 + mf * 128 + 128],
                            y2[:, k * RC:(k + 1) * RC],
                            start=(k == 0), stop=(k == 5))
                    nc.scalar.activation(out=ff1[:, mf * RC:(mf + 1) * RC],
                                         in_=ps[:], func=AF.Relu,
                                         bias=small["b1"][:, mf:mf + 1],
                                         scale=1.0)
                t3 = p3t.tile([128, 6 * RC], BF16)
                for m in range(6):
                    ps = p3ps.tile([128, RC], F32)
                    for k in range(16):
                        nc.tensor.matmul(
                            ps[:],
                            w2[:, k * 768 + m * 128:k * 768 + m * 128 + 128],
                            ff1[:, k * RC:(k + 1) * RC],
                            start=(k == 0), stop=(k == 15))
                    tb = p3s.tile([128, RC], BF16)
                    nc.scalar.activation(out=tb[:], in_=ps[:], func=AF.Identity,
                                         bias=small["b2"][:, m:m + 1], scale=1.0)
                    nc.vector.tensor_tensor(out=t3[:, m * RC:(m + 1) * RC],
                                            in0=tb[:],
                                            in1=y2[:, m * RC:(m + 1) * RC],
                                            op=ALU.add)
                layer_norm_T(t3, "ln3g", "ln3b",
                             lambda k: h3T[:, k * R + c * RC:k * R + (c + 1) * RC])

        # ---- P4: GroupFC -> logitsT ----
        with pool("p4d", bufs=12) as p4d, pool("p4o") as p4o, \
             pool("p4ps", bufs=2, space="PSUM") as p4ps:
            logitsT = p4o.tile([96, G * BL], F32)
            dupb = p4o.tile(list(d["dupb"].shape), BF16)
            nc.sync.dma_start(out=dupb, in_=d["dupb"])
            for g0 in range(0, G, 16):
                ng = min(16, G - g0)
                ps = p4ps.tile([96, 16 * BL], F32)
                for gi in range(ng):
                    g = g0 + gi
                    dup = p4d.tile([128, 6 * 96], BF16)
                    nc.sync.dma_start(out=dup, in_=d["dup"][g])
                    nc.tensor.matmul(ps[:, gi * BL:(gi + 1) * BL],
                                     dupb[:, g * 96:(g + 1) * 96],
                                     ones32[:], start=True, stop=False)
                    for k in range(6):
                        hsl = _ap(h3T[:, k * R + g:], [[100, BL]])
                        nc.tensor.matmul(ps[:, gi * BL:(gi + 1) * BL],
                                         dup[:, k * 96:(k + 1) * 96],
                                         hsl, start=False, stop=(k == 5))
                nc.vector.tensor_copy(out=logitsT[:, g0 * BL:(g0 + ng) * BL],
                                      in_=ps[:, 0:ng * BL])
            nc.sync.dma_start(out=out_d, in_=logitsT[:])


_CACHE = {}


def kernel(**inputs):
    f32 = lambda k: np.asarray(inputs[k], np.float32)
    x = f32("x")
    w_qkv, b_qkv = f32("w_qkv"), f32("b_qkv")
    w_attn_out, b_attn_out = f32("w_attn_out"), f32("b_attn_out")

    # host constant folding for the batch-independent query path
    t = 2.0 * f32("query_embed")
    mu = t.mean(-1, keepdims=True)
    va = ((t - mu) ** 2).mean(-1, keepdims=True)
    tgt0 = (t - mu) / np.sqrt(va + EPS) * f32("ln1_g") + f32("ln1_b")
    q = (tgt0 @ w_qkv[:, :D] + b_qkv[:D]) / np.sqrt(float(HD))
    bk = b_qkv[D:2 * D]
    qbk = np.stack([q[:, h * HD:(h + 1) * HD] @ bk[h * HD:(h + 1) * HD]
                    for h in range(H)], axis=1)
    bv = b_qkv[2 * D:]
    bao_eff = b_attn_out + bv @ w_attn_out   # softmax rows sum to 1

    col6 = lambda a: np.ascontiguousarray(a.reshape(6, 128).T)
    feed = {
        "wemb": _bf(f32("w_embed").reshape(16, 128, 768).transpose(1, 0, 2)
                    .reshape(128, -1)),
        "be": col6(f32("b_embed")),
        "wk": _bf(w_qkv[:, D:2 * D].reshape(6, 128, 768).transpose(1, 0, 2)
                  .reshape(128, -1)),
        "wv": _bf(w_qkv[:, 2 * D:].reshape(6, 128, 768).transpose(1, 0, 2)
                  .reshape(128, -1)),
        "wao": _bf(w_attn_out.reshape(8, 96, 768).transpose(1, 0, 2)
                   .reshape(96, -1)),
        "bao": col6(bao_eff),
        "w1": _bf(f32("w1").reshape(6, 128, 2048).transpose(1, 0, 2)
                  .reshape(128, -1)),
        "b1": np.ascontiguousarray(f32("b1").reshape(16, 128).T),
        "w2": _bf(f32("w2").reshape(16, 128, 768).transpose(1, 0, 2)
                  .reshape(128, -1)),
        "b2": col6(f32("b2")),
        "qT": _bf(q.T.reshape(8, 96, 100).transpose(1, 0, 2).reshape(96, -1)),
        "qbk": np.ascontiguousarray(qbk.astype(np.float32)),
        "tgt0": _bf(tgt0.T.reshape(6, 128, 100).transpose(1, 0, 2)
                    .reshape(128, -1)),
        "ln2g": col6(f32("ln2_g")), "ln2b": col6(f32("ln2_b")),
        "ln3g": col6(f32("ln3_g")), "ln3b": col6(f32("ln3_b")),
        "dup": _bf(f32("dup_pool").reshape(G, 6, 128, 96).transpose(0, 2, 1, 3)
                   .reshape(G, 128, 6 * 96)),
        "dupb": _bf(f32("dup_bias").reshape(1, -1)),
    }

    if "nc" not in _CACHE:
        _CACHE["nc"] = build_program()
    nc = _CACHE["nc"]

    # xr[core] axes: [c, col, k, p]; device wants [c, p, k, col]
    xr = x.reshape(NCORES, XCH, XCOLS, 16, 128)
    in_maps = []
    for core in range(NCORES):
        xT = xr[core].transpose(0, 3, 2, 1).reshape(XCH, 128, 16 * XCOLS)
        in_maps.append({**feed, "xT": _bf(xT)})

    _CACHE["in_maps"] = in_maps
    res = run_bass_kernel_spmd(nc, in_maps, list(range(NCORES)))
    outs = []
    for core in range(NCORES):
        lt = np.asarray(res.results[core]["logitsT"], np.float32)
        outs.append(lt.reshape(96, G, BL).transpose(2, 1, 0).reshape(BL, G * DF))
    return np.concatenate(outs, axis=0).astype(np.float32)


# revision 11
# speedup vs baseline: 1.1215x; 1.0633x over previous
"""Trainium2 Bass kernel for nn_MLDecoder (moe_routing).

Data-parallel over batch across 8 NeuronCores (32 batch rows/core, head params
replicated). Activations stay feature-major ("transposed"): C^T = W^T A^T via
matmul(out=C^T, lhsT=W(natural), rhs=A^T). Rows r = b*100+g (b-major). The
batch-independent query path (tgt0, q) is constant-folded on the host. All
matmuls bf16 with fp32 PSUM; LN stats via ones-matmuls; softmax without
max-subtraction (scores are O(1) for this head).
"""
import sys
sys.path.insert(0, "/opt/trn_rl_repo")

import numpy as np
import ml_dtypes

import concourse.bass as bass
from concourse import bacc
import concourse.tile as tile
import concourse.mybir as mybir
from concourse.bass import AP
from concourse.bass_utils import run_bass_kernel_spmd
from concourse.masks import make_identity

F32 = mybir.dt.float32
BF16 = mybir.dt.bfloat16
BF = ml_dtypes.bfloat16
AF = mybir.ActivationFunctionType
ALU = mybir.AluOpType
AX = mybir.AxisListType

B, S, C0 = 256, 49, 2048
D, F = 768, 2048
G, DF = 100, 96
H, HD = 8, 96
EPS = 1e-5
NCORES = 8
BL = B // NCORES          # 32 batch rows per core
R = BL * G                # 3200 rows (b,g) per core
RC = 400                  # row chunk = 4 b
NCHUNK = R // RC
XCH = 4                   # x col chunks (8 b each)
XCOLS = (BL // XCH) * S   # 392
PADS = 64                 # padded spatial stride
MCOLS = BL * PADS         # 2048 padded mem cols


def _bf(a):
    return np.ascontiguousarray(a.astype(BF))


def _ap(base, free_dims):
    """Replace the free dims of a (sliced) AP, keeping its partition dim."""
    return AP(tensor=base.tensor, offset=base.offset,
              ap=[base.ap[0]] + [list(fd) for fd in free_dims])


def build_program(skip_dupb=False, ln_triv=False):
    nc = bacc.Bacc("TRN2", target_bir_lowering=False, debug=False,
                   num_devices=NCORES)
    d = {}

    def din(name, shape, dt):
        d[name] = nc.dram_tensor(name, list(shape), dt, kind="ExternalInput").ap()

    din("xT", (XCH, 128, 16 * XCOLS), BF16)
    din("wemb", (128, 16 * 768), BF16)
    din("be", (128, 6), F32)
    din("wk", (128, 6 * 768), BF16)
    din("wv", (128, 6 * 768), BF16)
    din("wao", (96, 8 * 768), BF16)
    din("bao", (128, 6), F32)
    din("w1", (128, 6 * 2048), BF16)
    din("b1", (128, 16), F32)
    din("w2", (128, 16 * 768), BF16)
    din("b2", (128, 6), F32)
    din("qT", (96, 8 * 100), BF16)
    din("qbk", (100, 8), F32)
    din("tgt0", (128, 6 * 100), BF16)
    din("ln2g", (128, 6), F32)
    din("ln2b", (128, 6), F32)
    din("ln3g", (128, 6), F32)
    din("ln3b", (128, 6), F32)
    din("dup", (100, 128, 6 * 96), BF16)
    din("dupb", (1, G * 96), BF16)
    out_d = nc.dram_tensor("logitsT", [96, G * BL], F32,
                           kind="ExternalOutput").ap()

    with tile.TileContext(nc) as tc:
        build_kernel(tc, d, out_d, skip_dupb, ln_triv)
    nc.compile()
    return nc


def build_kernel(tc, d, out_d, skip_dupb=False, ln_triv=False):
    nc = tc.nc

    def pool(name, bufs=1, space="SBUF"):
        return tc.tile_pool(name=name, bufs=bufs, space=space)

    with pool("resident") as res, pool("h3pool") as h3p, pool("oTpool") as oTp:
        ident = res.tile([128, 128], BF16)
        make_identity(nc, ident[:])
        ones_col = res.tile([128, 1], BF16)
        nc.vector.memset(ones_col[:], 1.0)
        ones_row = res.tile([1, 128], BF16)
        nc.vector.memset(ones_row[:], 1.0)
        ones32 = res.tile([1, BL], BF16)
        nc.vector.memset(ones32[:], 1.0)
        eps_t = res.tile([1, 1], F32)
        nc.vector.memset(eps_t[:], EPS)

        small = {}
        for name in ["be", "bao", "b1", "b2", "qT", "qbk", "tgt0",
                     "ln2g", "ln2b", "ln3g", "ln3b"]:
            t = res.tile(list(d[name].shape), d[name].dtype, tag=name)
            nc.sync.dma_start(out=t, in_=d[name])
            small[name] = t

        h3T = h3p.tile([128, 6 * R], BF16)
        oT = oTp.tile([96, 8 * R], BF16)

        with pool("memTpool") as memp:
            memT = memp.tile([128, 6 * MCOLS], BF16)

            # ---- P0: mem^T = relu(We^T x^T + be), written b-padded ----
            with pool("p0w") as p0w, pool("p0x", bufs=2) as p0x, \
                 pool("p0ps", bufs=3, space="PSUM") as p0ps:
                wemb = p0w.tile([128, 16 * 768], BF16)
                nc.sync.dma_start(out=wemb, in_=d["wemb"])
                for c in range(XCH):
                    xt = p0x.tile([128, 16 * XCOLS], BF16)
                    nc.sync.dma_start(out=xt, in_=d["xT"][c])
                    for m in range(6):
                        ps = p0ps.tile([128, XCOLS], F32)
                        for k in range(16):
                            nc.tensor.matmul(
                                ps[:],
                                wemb[:, k * 768 + m * 128:k * 768 + m * 128 + 128],
                                xt[:, k * XCOLS:(k + 1) * XCOLS],
                                start=(k == 0), stop=(k == 15))
                        dst = _ap(memT[:, m * MCOLS + c * 8 * PADS:],
                                  [[PADS, 8], [1, S]])
                        src = _ap(ps[:], [[S, 8], [1, S]])
                        nc.scalar.activation(out=dst, in_=src, func=AF.Relu,
                                             bias=small["be"][:, m:m + 1],
                                             scale=1.0)

            # ---- P1: K^T (head-major, b-padded) and V (rows padded) ----
            with pool("kvpool") as kvp:
                KT = kvp.tile([96, 8 * MCOLS], BF16)
                Vp = kvp.tile([128, 16 * 768], BF16)
                with pool("p1w") as p1w, \
                     pool("p1ps", bufs=3, space="PSUM") as p1ps:
                    wk = p1w.tile([128, 6 * 768], BF16)
                    nc.sync.dma_start(out=wk, in_=d["wk"])
                    wv = p1w.tile([128, 6 * 768], BF16)
                    nc.sync.dma_start(out=wv, in_=d["wv"])
                    for h in range(H):
                        for c in range(XCH):
                            ps = p1ps.tile([96, XCOLS], F32)
                            for k in range(6):
                                rhs = _ap(memT[:, k * MCOLS + c * 8 * PADS:],
                                          [[PADS, 8], [1, S]])
                                nc.tensor.matmul(
                                    ps[:],
                                    wk[:, k * 768 + h * 96:k * 768 + h * 96 + 96],
                                    rhs, start=(k == 0), stop=(k == 5))
                            dst = _ap(KT[:, h * MCOLS + c * 8 * PADS:],
                                      [[PADS, 8], [1, S]])
                            nc.vector.tensor_copy(
                                out=dst, in_=_ap(ps[:], [[S, 8], [1, S]]))
                    for t in range(16):
                        ps = p1ps.tile([128, 768], F32)
                        for sub in range(2):
                            n0, n1 = sub * 512, min(768, (sub + 1) * 512)
                            for k in range(6):
                                nc.tensor.matmul(
                                    ps[:, n0:n1],
                                    memT[:, k * MCOLS + t * 128:
                                         k * MCOLS + t * 128 + 128],
                                    wv[:, k * 768 + n0:k * 768 + n1],
                                    start=(k == 0), stop=(k == 5))
                        nc.vector.tensor_copy(out=Vp[:, t * 768:(t + 1) * 768],
                                              in_=ps[:])

                # ---- P2: attention ----
                with pool("p2a", bufs=2) as p2a, pool("p2s", bufs=3) as p2s, \
                     pool("p2ps", bufs=2, space="PSUM") as psc, \
                     pool("p2pt", bufs=2, space="PSUM") as pst, \
                     pool("p2po", bufs=2, space="PSUM") as pso:
                    for bg in range(4):
                        attnT = p2a.tile([128, 8 * 400], BF16)
                        for h in range(H):
                            ps = psc.tile([100, 8 * S], F32)
                            rhs = _ap(KT[:, h * MCOLS + bg * 8 * PADS:],
                                      [[PADS, 8], [1, S]])
                            nc.tensor.matmul(ps[:],
                                             small["qT"][:, h * 100:(h + 1) * 100],
                                             rhs, start=True, stop=True)
                            # exp into 64-padded slots (pads hold garbage,
                            # excluded by every later access pattern)
                            att = p2s.tile([100, 8 * PADS], BF16)
                            nc.scalar.activation(out=_ap(att[:], [[PADS, 8], [1, S]]),
                                                 in_=ps[:],
                                                 func=AF.Exp,
                                                 bias=small["qbk"][:, h:h + 1],
                                                 scale=1.0)
                            sums = p2s.tile([100, 8], F32)
                            nc.vector.reduce_sum(out=sums[:],
                                                 in_=_ap(att[:], [[PADS, 8], [1, S]]),
                                                 axis=AX.X)
                            inv = p2s.tile([100, 8], F32)
                            nc.vector.reciprocal(out=inv[:], in_=sums[:])
                            attn = p2s.tile([100, 8 * PADS], BF16)
                            nc.vector.tensor_tensor(
                                out=_ap(attn[:], [[PADS, 8], [1, S]]),
                                in0=_ap(att[:], [[PADS, 8], [1, S]]),
                                in1=_ap(inv[:], [[1, 8], [0, S]]),
                                op=ALU.mult)
                            for pr in range(4):
                                pt = pst.tile([128, 100], BF16)
                                nc.tensor.transpose(
                                    pt[:], attn[:, pr * 128:(pr + 1) * 128],
                                    ident[0:100, 0:100])
                                nc.vector.tensor_copy(
                                    out=attnT[:, h * 400 + pr * 100:
                                              h * 400 + pr * 100 + 100],
                                    in_=pt[:])
                        for lb in range(8):
                            b = bg * 8 + lb
                            po = pso.tile([96, 1024], F32)
                            for h in range(H):
                                vsl = Vp[(lb % 2) * 64:(lb % 2) * 64 + S,
                                         (b // 2) * 768 + h * 96:
                                         (b // 2) * 768 + h * 96 + 96]
                                nc.tensor.matmul(
                                    po[:, h * 128:h * 128 + 100], vsl,
                                    attnT[(lb % 2) * 64:(lb % 2) * 64 + S,
                                          h * 400 + (lb // 2) * 100:
                                          h * 400 + (lb // 2) * 100 + 100],
                                    start=True, stop=True)
                            dst = _ap(oT[:, b * 100:], [[R, 8], [1, 100]])
                            nc.vector.tensor_copy(
                                out=dst, in_=_ap(po[:], [[128, 8], [1, 100]]))

        # ---- P3: attn_out + LN2 + FFN + LN3 -> h3T ----
        with pool("p3w") as p3w, pool("p3t") as p3t, \
             pool("p3f") as p3f, pool("p3s", bufs=2) as p3s, \
             pool("p3ps", bufs=4, space="PSUM") as p3ps, \
             pool("p3st", space="PSUM") as p3st, \
             pool("p3ab", space="PSUM") as p3ab:
            wao = p3w.tile([96, 8 * 768], BF16)
            nc.sync.dma_start(out=wao, in_=d["wao"])
            w1 = p3w.tile([128, 6 * 2048], BF16)
            nc.sync.dma_start(out=w1, in_=d["w1"])
            w2 = p3w.tile([128, 16 * 768], BF16)
            nc.sync.dma_start(out=w2, in_=d["w2"])

            def layer_norm_T(xin, gname, bname, yout):
                sq = p3f.tile([128, 6 * RC], BF16)
                nc.scalar.square(out=sq[:], in_=xin[:])
                s1 = p3st.tile([1, RC], F32)
                s2 = p3st.tile([1, RC], F32)
                for k in range(6):
                    nc.tensor.matmul(s1[:], ones_col[:],
                                     xin[:, k * RC:(k + 1) * RC],
                                     start=(k == 0), stop=(k == 5))
                for k in range(6):
                    nc.tensor.matmul(s2[:], ones_col[:],
                                     sq[:, k * RC:(k + 1) * RC],
                                     start=(k == 0), stop=(k == 5))
                mean = p3f.tile([1, RC], F32)
                nc.vector.tensor_scalar_mul(out=mean[:], in0=s1[:],
                                            scalar1=1.0 / D)
                var = p3f.tile([1, RC], F32)
                nc.vector.tensor_scalar_mul(out=var[:], in0=s2[:],
                                            scalar1=1.0 / D)
                msq = p3f.tile([1, RC], F32)
                nc.vector.tensor_tensor(out=msq[:], in0=mean[:], in1=mean[:],
                                        op=ALU.mult)
                nc.vector.tensor_tensor(out=var[:], in0=var[:], in1=msq[:],
                                        op=ALU.subtract)
                sd = p3f.tile([1, RC], F32)
                nc.scalar.activation(out=sd[:], in_=var[:], func=AF.Sqrt,
                                     bias=eps_t[:], scale=1.0)
                rstd = p3f.tile([1, RC], F32)
                nc.vector.reciprocal(out=rstd[:], in_=sd[:])
                nmr = p3f.tile([1, RC], F32)
                nc.vector.tensor_tensor(out=nmr[:], in0=mean[:], in1=rstd[:],
                                        op=ALU.mult)
                rstd_b = p3f.tile([1, RC], BF16)
                nc.vector.tensor_copy(out=rstd_b[:], in_=rstd[:])
                nmr_b = p3f.tile([1, RC], BF16)
                nc.vector.tensor_scalar_mul(out=nmr_b[:], in0=nmr[:], scalar1=-1.0)
                pa = p3ab.tile([128, RC], F32)
                nc.tensor.matmul(pa[:], ones_row[:], rstd_b[:],
                                 start=True, stop=True)
                pb = p3ab.tile([128, RC], F32)
                nc.tensor.matmul(pb[:], ones_row[:], nmr_b[:],
                                 start=True, stop=True)
                gv, bv = small[gname], small[bname]
                for k in range(6):
                    u = p3s.tile([128, RC], F32)
                    nc.vector.tensor_tensor(out=u[:],
                                            in0=xin[:, k * RC:(k + 1) * RC],
                                            in1=pa[:], op=ALU.mult)
                    if ln_triv:
                        nc.vector.tensor_tensor(out=yout(k), in0=u[:],
                                                in1=pb[:], op=ALU.add)
                    else:
                        nc.vector.tensor_tensor(out=u[:], in0=u[:], in1=pb[:],
                                                op=ALU.add)
                        nc.vector.tensor_scalar(out=yout(k), in0=u[:],
                                                scalar1=gv[:, k:k + 1],
                                                scalar2=bv[:, k:k + 1],
                                                op0=ALU.mult, op1=ALU.add)

            for c in range(NCHUNK):
                t2 = p3t.tile([128, 6 * RC], BF16)
                for m in range(6):
                    ps = p3ps.tile([128, RC], F32)
                    for kh in range(H):
                        nc.tensor.matmul(
                            ps[:],
                            wao[:, kh * 768 + m * 128:kh * 768 + m * 128 + 128],
                            oT[:, kh * R + c * RC:kh * R + (c + 1) * RC],
                            start=(kh == 0), stop=(kh == 7))
                    ta = p3s.tile([128, RC], BF16)
                    nc.scalar.activation(out=ta[:], in_=ps[:], func=AF.Identity,
                                         bias=small["bao"][:, m:m + 1], scale=1.0)
                    tg = small["tgt0"][:, m * 100:(m + 1) * 100]
                    nc.vector.tensor_tensor(out=t2[:, m * RC:(m + 1) * RC],
                                            in0=ta[:],
                                            in1=_ap(tg, [[0, 4], [1, 100]]),
                                            op=ALU.add)
                y2 = p3t.tile([128, 6 * RC], BF16)
                layer_norm_T(t2, "ln2g", "ln2b",
                             lambda k: y2[:, k * RC:(k + 1) * RC])
                ff1 = p3f.tile([128, 16 * RC], BF16)
                for mf in range(16):
                    ps = p3ps.tile([128, RC], F32)
                    for k in range(6):
                        nc.tensor.matmul(
                            ps[:],
                            w1[:, k * 2048 + mf * 128:k * 2048 + mf * 128 + 128],
                            y2[:, k * RC:(k + 1) * RC],
                            start=(k == 0), stop=(k == 5))
                    nc.scalar.activation(out=ff1[:, mf * RC:(mf + 1) * RC],
                                         in_=ps[:], func=AF.Relu,
                                         bias=small["b1"][:, mf:mf + 1],
                                         scale=1.0)
                t3 = p3t.tile([128, 6 * RC], BF16)
                for m in range(6):
                    ps = p3ps.tile([128, RC], F32)
                    for k in range(16):
                        nc.tensor.matmul(
                            ps[:],
                            w2[:, k * 768 + m * 128:k * 768 + m * 128 + 128],
                            ff1[:, k * RC:(k + 1) * RC],
                            start=(k == 0), stop=(k == 15))
                    tb = p3s.tile([128, RC], BF16)
                    nc.scalar.activation(out=tb[:], in_=ps[:], func=AF.Identity,
                                         bias=small["b2"][:, m:m + 1], scale=1.0)
                    nc.vector.tensor_tensor(out=t3[:, m * RC:(m + 1) * RC],
                                            in0=tb[:],
                                            in1=y2[:, m * RC:(m + 1) * RC],
                                            op=ALU.add)
                layer_norm_T(t3, "ln3g", "ln3b",
                             lambda k: h3T[:, k * R + c * RC:k * R + (c + 1) * RC])

        # ---- P4: GroupFC -> logitsT ----
        with pool("p4d", bufs=12) as p4d, pool("p4o") as p4o, \
             pool("p4ps", bufs=2, space="PSUM") as p4ps:
            logitsT = p4o.tile([96, G * BL], F32)
            dupb = p4o.tile(list(d["dupb"].shape), BF16)
            nc.sync.dma_start(out=dupb, in_=d["dupb"])
            for g0 in range(0, G, 16):
                ng = min(16, G - g0)
                ps = p4ps.tile([96, 16 * BL], F32)
                for gi in range(ng):
                    g = g0 + gi
                    dup = p4d.tile([128, 6 * 96], BF16)
                    nc.sync.dma_start(out=dup, in_=d["dup"][g])
                    if not skip_dupb:
                        nc.tensor.matmul(ps[:, gi * BL:(gi + 1) * BL],
                                         dupb[:, g * 96:(g + 1) * 96],
                                         ones32[:], start=True, stop=False)
                    for k in range(6):
                        hsl = _ap(h3T[:, k * R + g:], [[100, BL]])
                        nc.tensor.matmul(ps[:, gi * BL:(gi + 1) * BL],
                                         dup[:, k * 96:(k + 1) * 96],
                                         hsl, start=(skip_dupb and k == 0),
                                         stop=(k == 5))
                nc.vector.tensor_copy(out=logitsT[:, g0 * BL:(g0 + ng) * BL],
                                      in_=ps[:, 0:ng * BL])
            nc.sync.dma_start(out=out_d, in_=logitsT[:])


_CACHE = {}


def kernel(**inputs):
    f32 = lambda k: np.asarray(inputs[k], np.float32)
    x = f32("x")
    w_qkv, b_qkv = f32("w_qkv"), f32("b_qkv")
    w_attn_out, b_attn_out = f32("w_attn_out"), f32("b_attn_out")

    # host constant folding for the batch-independent query path
    t = 2.0 * f32("query_embed")
    mu = t.mean(-1, keepdims=True)
    va = ((t - mu) ** 2).mean(-1, keepdims=True)
    tgt0 = (t - mu) / np.sqrt(va + EPS) * f32("ln1_g") + f32("ln1_b")
    q = (tgt0 @ w_qkv[:, :D] + b_qkv[:D]) / np.sqrt(float(HD))
    bk = b_qkv[D:2 * D]
    qbk = np.stack([q[:, h * HD:(h + 1) * HD] @ bk[h * HD:(h + 1) * HD]
                    for h in range(H)], axis=1)
    bv = b_qkv[2 * D:]
    bao_eff = b_attn_out + bv @ w_attn_out   # softmax rows sum to 1

    col6 = lambda a: np.ascontiguousarray(a.reshape(6, 128).T)
    feed = {
        "wemb": _bf(f32("w_embed").reshape(16, 128, 768).transpose(1, 0, 2)
                    .reshape(128, -1)),
        "be": col6(f32("b_embed")),
        "wk": _bf(w_qkv[:, D:2 * D].reshape(6, 128, 768).transpose(1, 0, 2)
                  .reshape(128, -1)),
        "wv": _bf(w_qkv[:, 2 * D:].reshape(6, 128, 768).transpose(1, 0, 2)
                  .reshape(128, -1)),
        "wao": _bf(w_attn_out.reshape(8, 96, 768).transpose(1, 0, 2)
                   .reshape(96, -1)),
        "bao": col6(bao_eff),
        "w1": _bf(f32("w1").reshape(6, 128, 2048).transpose(1, 0, 2)
                  .reshape(128, -1)),
        "b1": np.ascontiguousarray(f32("b1").reshape(16, 128).T),
        "w2": _bf(f32("w2").reshape(16, 128, 768).transpose(1, 0, 2)
                  .reshape(128, -1)),
        "b2": col6(f32("b2")),
        "qT": _bf(q.T.reshape(8, 96, 100).transpose(1, 0, 2).reshape(96, -1)),
        "qbk": np.ascontiguousarray(qbk.astype(np.float32)),
        "tgt0": _bf(tgt0.T.reshape(6, 128, 100).transpose(1, 0, 2)
                    .reshape(128, -1)),
        "ln2g": col6(f32("ln2_g")), "ln2b": col6(f32("ln2_b")),
        "ln3g": col6(f32("ln3_g")), "ln3b": col6(f32("ln3_b")),
        "dup": _bf(f32("dup_pool").reshape(G, 6, 128, 96).transpose(0, 2, 1, 3)
                   .reshape(G, 128, 6 * 96)),
        "dupb": _bf(f32("dup_bias").reshape(1, -1)),
    }

    skip_dupb = bool(np.all(f32("dup_bias") == 0.0))
    ln_triv = bool(np.all(f32("ln2_g") == 1.0) and np.all(f32("ln2_b") == 0.0)
                   and np.all(f32("ln3_g") == 1.0) and np.all(f32("ln3_b") == 0.0))
    key = ("nc", skip_dupb, ln_triv)
    if key not in _CACHE:
        _CACHE[key] = build_program(skip_dupb, ln_triv)
    nc = _CACHE[key]
    _CACHE["nc"] = nc

    # xr[core] axes: [c, col, k, p]; device wants [c, p, k, col]
    xr = x.reshape(NCORES, XCH, XCOLS, 16, 128)
    in_maps = []
    for core in range(NCORES):
        xT = xr[core].transpose(0, 3, 2, 1).reshape(XCH, 128, 16 * XCOLS)
        in_maps.append({**feed, "xT": _bf(xT)})

    _CACHE["in_maps"] = in_maps
    res = run_bass_kernel_spmd(nc, in_maps, list(range(NCORES)))
    outs = []
    for core in range(NCORES):
        lt = np.asarray(res.results[core]["logitsT"], np.float32)
        outs.append(lt.reshape(96, G, BL).transpose(2, 1, 0).reshape(BL, G * DF))
    return np.concatenate(outs, axis=0).astype(np.float32)


# revision 12
# speedup vs baseline: 1.1913x; 1.0622x over previous
"""Trainium2 Bass kernel for nn_MLDecoder (moe_routing).

Data-parallel over batch across 8 NeuronCores (32 batch rows/core, head params
replicated). Activations stay feature-major ("transposed"): C^T = W^T A^T via
matmul(out=C^T, lhsT=W(natural), rhs=A^T). Rows r = b*100+g (b-major). The
batch-independent query path (tgt0, q) is constant-folded on the host. All
matmuls bf16 with fp32 PSUM; LN stats via ones-matmuls; softmax without
max-subtraction (scores are O(1) for this head).
"""
import sys
sys.path.insert(0, "/opt/trn_rl_repo")

import numpy as np
import ml_dtypes

import concourse.bass as bass
from concourse import bacc
import concourse.tile as tile
import concourse.mybir as mybir
from concourse.bass import AP
from concourse.bass_utils import run_bass_kernel_spmd
from concourse.masks import make_identity

F32 = mybir.dt.float32
BF16 = mybir.dt.bfloat16
BF = ml_dtypes.bfloat16
AF = mybir.ActivationFunctionType
ALU = mybir.AluOpType
AX = mybir.AxisListType

B, S, C0 = 256, 49, 2048
D, F = 768, 2048
G, DF = 100, 96
H, HD = 8, 96
EPS = 1e-5
NCORES = 8
BL = B // NCORES          # 32 batch rows per core
R = BL * G                # 3200 rows (b,g) per core
RC = 400                  # row chunk = 4 b
NCHUNK = R // RC
XCH = 4                   # x col chunks (8 b each)
XCOLS = (BL // XCH) * S   # 392
PADS = 64                 # padded spatial stride
MCOLS = BL * PADS         # 2048 padded mem cols


def _bf(a):
    return np.ascontiguousarray(a.astype(BF))


def _ap(base, free_dims):
    """Replace the free dims of a (sliced) AP, keeping its partition dim."""
    return AP(tensor=base.tensor, offset=base.offset,
              ap=[base.ap[0]] + [list(fd) for fd in free_dims])


def build_program(skip_dupb=False, ln_triv=False, ffn_triv=False):
    nc = bacc.Bacc("TRN2", target_bir_lowering=False, debug=False,
                   num_devices=NCORES)
    d = {}

    def din(name, shape, dt):
        d[name] = nc.dram_tensor(name, list(shape), dt, kind="ExternalInput").ap()

    din("xT", (XCH, 128, 16 * XCOLS), BF16)
    din("wemb", (128, 16 * 768), BF16)
    din("be", (128, 6), F32)
    din("wk", (128, 6 * 768), BF16)
    din("wv", (128, 6 * 768), BF16)
    din("wao", (96, 8 * 768), BF16)
    din("bao", (128, 6), F32)
    din("w1", (128, 6 * 2048), BF16)
    din("b1", (128, 16), F32)
    din("w2", (128, 16 * 768), BF16)
    din("b2", (128, 6), F32)
    din("qT", (96, 8 * 100), BF16)
    din("qbk", (100, 8), F32)
    din("tgt0", (128, 6 * 100), BF16)
    din("ln2g", (128, 6), F32)
    din("ln2b", (128, 6), F32)
    din("ln3g", (128, 6), F32)
    din("ln3b", (128, 6), F32)
    din("dup", (100, 128, 6 * 96), BF16)
    din("dupb", (1, G * 96), BF16)
    out_d = nc.dram_tensor("logitsT", [96, G * BL], F32,
                           kind="ExternalOutput").ap()

    with tile.TileContext(nc) as tc:
        build_kernel(tc, d, out_d, skip_dupb, ln_triv, ffn_triv)
    nc.compile()
    return nc


def build_kernel(tc, d, out_d, skip_dupb=False, ln_triv=False, ffn_triv=False):
    nc = tc.nc

    def pool(name, bufs=1, space="SBUF"):
        return tc.tile_pool(name=name, bufs=bufs, space=space)

    with pool("resident") as res, pool("h3pool") as h3p, pool("oTpool") as oTp:
        ident = res.tile([128, 128], BF16)
        make_identity(nc, ident[:])
        ones_col = res.tile([128, 1], BF16)
        nc.vector.memset(ones_col[:], 1.0)
        ones_row = res.tile([1, 128], BF16)
        nc.vector.memset(ones_row[:], 1.0)
        ones32 = res.tile([1, BL], BF16)
        nc.vector.memset(ones32[:], 1.0)
        eps_t = res.tile([1, 1], F32)
        nc.vector.memset(eps_t[:], EPS)

        small = {}
        for name in ["be", "bao", "b1", "b2", "qT", "qbk", "tgt0",
                     "ln2g", "ln2b", "ln3g", "ln3b"]:
            t = res.tile(list(d[name].shape), d[name].dtype, tag=name)
            nc.sync.dma_start(out=t, in_=d[name])
            small[name] = t

        h3T = h3p.tile([128, 6 * R], BF16)
        oT = oTp.tile([96, 8 * R], BF16)

        with pool("memTpool") as memp:
            memT = memp.tile([128, 6 * MCOLS], BF16)

            # ---- P0: mem^T = relu(We^T x^T + be), written b-padded ----
            with pool("p0w") as p0w, pool("p0x", bufs=2) as p0x, \
                 pool("p0ps", bufs=3, space="PSUM") as p0ps:
                wemb = p0w.tile([128, 16 * 768], BF16)
                nc.sync.dma_start(out=wemb, in_=d["wemb"])
                for c in range(XCH):
                    xt = p0x.tile([128, 16 * XCOLS], BF16)
                    nc.sync.dma_start(out=xt, in_=d["xT"][c])
                    for m in range(6):
                        ps = p0ps.tile([128, XCOLS], F32)
                        for k in range(16):
                            nc.tensor.matmul(
                                ps[:],
                                wemb[:, k * 768 + m * 128:k * 768 + m * 128 + 128],
                                xt[:, k * XCOLS:(k + 1) * XCOLS],
                                start=(k == 0), stop=(k == 15))
                        dst = _ap(memT[:, m * MCOLS + c * 8 * PADS:],
                                  [[PADS, 8], [1, S]])
                        src = _ap(ps[:], [[S, 8], [1, S]])
                        nc.scalar.activation(out=dst, in_=src, func=AF.Relu,
                                             bias=small["be"][:, m:m + 1],
                                             scale=1.0)

            # ---- P1: K^T (head-major, b-padded) and V (rows padded) ----
            with pool("kvpool") as kvp:
                KT = kvp.tile([96, 8 * MCOLS], BF16)
                Vp = kvp.tile([128, 16 * 768], BF16)
                with pool("p1w") as p1w, \
                     pool("p1ps", bufs=3, space="PSUM") as p1ps:
                    wk = p1w.tile([128, 6 * 768], BF16)
                    nc.sync.dma_start(out=wk, in_=d["wk"])
                    wv = p1w.tile([128, 6 * 768], BF16)
                    nc.sync.dma_start(out=wv, in_=d["wv"])
                    for h in range(H):
                        for c in range(XCH):
                            ps = p1ps.tile([96, XCOLS], F32)
                            for k in range(6):
                                rhs = _ap(memT[:, k * MCOLS + c * 8 * PADS:],
                                          [[PADS, 8], [1, S]])
                                nc.tensor.matmul(
                                    ps[:],
                                    wk[:, k * 768 + h * 96:k * 768 + h * 96 + 96],
                                    rhs, start=(k == 0), stop=(k == 5))
                            dst = _ap(KT[:, h * MCOLS + c * 8 * PADS:],
                                      [[PADS, 8], [1, S]])
                            nc.vector.tensor_copy(
                                out=dst, in_=_ap(ps[:], [[S, 8], [1, S]]))
                    for t in range(16):
                        ps = p1ps.tile([128, 768], F32)
                        for sub in range(2):
                            n0, n1 = sub * 512, min(768, (sub + 1) * 512)
                            for k in range(6):
                                nc.tensor.matmul(
                                    ps[:, n0:n1],
                                    memT[:, k * MCOLS + t * 128:
                                         k * MCOLS + t * 128 + 128],
                                    wv[:, k * 768 + n0:k * 768 + n1],
                                    start=(k == 0), stop=(k == 5))
                        nc.vector.tensor_copy(out=Vp[:, t * 768:(t + 1) * 768],
                                              in_=ps[:])

                # ---- P2: attention ----
                with pool("p2a", bufs=2) as p2a, pool("p2s", bufs=3) as p2s, \
                     pool("p2ps", bufs=2, space="PSUM") as psc, \
                     pool("p2pt", bufs=2, space="PSUM") as pst, \
                     pool("p2po", bufs=2, space="PSUM") as pso:
                    for bg in range(4):
                        attnT = p2a.tile([128, 8 * 400], BF16)
                        for h in range(H):
                            ps = psc.tile([100, 8 * S], F32)
                            rhs = _ap(KT[:, h * MCOLS + bg * 8 * PADS:],
                                      [[PADS, 8], [1, S]])
                            nc.tensor.matmul(ps[:],
                                             small["qT"][:, h * 100:(h + 1) * 100],
                                             rhs, start=True, stop=True)
                            # exp into 64-padded slots (pads hold garbage,
                            # excluded by every later access pattern)
                            att = p2s.tile([100, 8 * PADS], BF16)
                            nc.scalar.activation(out=_ap(att[:], [[PADS, 8], [1, S]]),
                                                 in_=ps[:],
                                                 func=AF.Exp,
                                                 bias=small["qbk"][:, h:h + 1],
                                                 scale=1.0)
                            sums = p2s.tile([100, 8], F32)
                            nc.vector.reduce_sum(out=sums[:],
                                                 in_=_ap(att[:], [[PADS, 8], [1, S]]),
                                                 axis=AX.X)
                            inv = p2s.tile([100, 8], F32)
                            nc.vector.reciprocal(out=inv[:], in_=sums[:])
                            attn = p2s.tile([100, 8 * PADS], BF16)
                            nc.vector.tensor_tensor(
                                out=_ap(attn[:], [[PADS, 8], [1, S]]),
                                in0=_ap(att[:], [[PADS, 8], [1, S]]),
                                in1=_ap(inv[:], [[1, 8], [0, S]]),
                                op=ALU.mult)
                            for pr in range(4):
                                pt = pst.tile([128, 100], BF16)
                                nc.tensor.transpose(
                                    pt[:], attn[:, pr * 128:(pr + 1) * 128],
                                    ident[0:100, 0:100])
                                nc.vector.tensor_copy(
                                    out=attnT[:, h * 400 + pr * 100:
                                              h * 400 + pr * 100 + 100],
                                    in_=pt[:])
                        for lb in range(8):
                            b = bg * 8 + lb
                            po = pso.tile([96, 1024], F32)
                            for h in range(H):
                                vsl = Vp[(lb % 2) * 64:(lb % 2) * 64 + S,
                                         (b // 2) * 768 + h * 96:
                                         (b // 2) * 768 + h * 96 + 96]
                                nc.tensor.matmul(
                                    po[:, h * 128:h * 128 + 100], vsl,
                                    attnT[(lb % 2) * 64:(lb % 2) * 64 + S,
                                          h * 400 + (lb // 2) * 100:
                                          h * 400 + (lb // 2) * 100 + 100],
                                    start=True, stop=True)
                            dst = _ap(oT[:, b * 100:], [[R, 8], [1, 100]])
                            nc.vector.tensor_copy(
                                out=dst, in_=_ap(po[:], [[128, 8], [1, 100]]))

        # ---- P3: attn_out + LN2 + FFN + LN3 -> h3T ----
        with pool("p3w") as p3w, pool("p3t") as p3t, \
             pool("p3f") as p3f, pool("p3s", bufs=2) as p3s, \
             pool("p3ps", bufs=4, space="PSUM") as p3ps, \
             pool("p3st", space="PSUM") as p3st, \
             pool("p3ab", space="PSUM") as p3ab:
            wao = p3w.tile([96, 8 * 768], BF16)
            nc.sync.dma_start(out=wao, in_=d["wao"])
            w1 = p3w.tile([128, 6 * 2048], BF16)
            nc.sync.dma_start(out=w1, in_=d["w1"])
            w2 = p3w.tile([128, 16 * 768], BF16)
            nc.sync.dma_start(out=w2, in_=d["w2"])

            def layer_norm_T(xin, gname, bname, yout):
                sq = p3f.tile([128, 6 * RC], BF16)
                nc.scalar.square(out=sq[:], in_=xin[:])
                s1 = p3st.tile([1, RC], F32)
                s2 = p3st.tile([1, RC], F32)
                for k in range(6):
                    nc.tensor.matmul(s1[:], ones_col[:],
                                     xin[:, k * RC:(k + 1) * RC],
                                     start=(k == 0), stop=(k == 5))
                for k in range(6):
                    nc.tensor.matmul(s2[:], ones_col[:],
                                     sq[:, k * RC:(k + 1) * RC],
                                     start=(k == 0), stop=(k == 5))
                mean = p3f.tile([1, RC], F32)
                nc.vector.tensor_scalar_mul(out=mean[:], in0=s1[:],
                                            scalar1=1.0 / D)
                var = p3f.tile([1, RC], F32)
                nc.vector.tensor_scalar_mul(out=var[:], in0=s2[:],
                                            scalar1=1.0 / D)
                msq = p3f.tile([1, RC], F32)
                nc.vector.tensor_tensor(out=msq[:], in0=mean[:], in1=mean[:],
                                        op=ALU.mult)
                nc.vector.tensor_tensor(out=var[:], in0=var[:], in1=msq[:],
                                        op=ALU.subtract)
                sd = p3f.tile([1, RC], F32)
                nc.scalar.activation(out=sd[:], in_=var[:], func=AF.Sqrt,
                                     bias=eps_t[:], scale=1.0)
                rstd = p3f.tile([1, RC], F32)
                nc.vector.reciprocal(out=rstd[:], in_=sd[:])
                nmr = p3f.tile([1, RC], F32)
                nc.vector.tensor_tensor(out=nmr[:], in0=mean[:], in1=rstd[:],
                                        op=ALU.mult)
                rstd_b = p3f.tile([1, RC], BF16)
                nc.vector.tensor_copy(out=rstd_b[:], in_=rstd[:])
                nmr_b = p3f.tile([1, RC], BF16)
                nc.vector.tensor_scalar_mul(out=nmr_b[:], in0=nmr[:], scalar1=-1.0)
                pa = p3ab.tile([128, RC], F32)
                nc.tensor.matmul(pa[:], ones_row[:], rstd_b[:],
                                 start=True, stop=True)
                pb = p3ab.tile([128, RC], F32)
                nc.tensor.matmul(pb[:], ones_row[:], nmr_b[:],
                                 start=True, stop=True)
                gv, bv = small[gname], small[bname]
                for k in range(6):
                    u = p3s.tile([128, RC], F32)
                    nc.vector.tensor_tensor(out=u[:],
                                            in0=xin[:, k * RC:(k + 1) * RC],
                                            in1=pa[:], op=ALU.mult)
                    if ln_triv:
                        nc.vector.tensor_tensor(out=yout(k), in0=u[:],
                                                in1=pb[:], op=ALU.add)
                    else:
                        nc.vector.tensor_tensor(out=u[:], in0=u[:], in1=pb[:],
                                                op=ALU.add)
                        nc.vector.tensor_scalar(out=yout(k), in0=u[:],
                                                scalar1=gv[:, k:k + 1],
                                                scalar2=bv[:, k:k + 1],
                                                op0=ALU.mult, op1=ALU.add)

            for c in range(NCHUNK):
                t2 = p3t.tile([128, 6 * RC], BF16)
                for m in range(6):
                    ps = p3ps.tile([128, RC], F32)
                    for kh in range(H):
                        nc.tensor.matmul(
                            ps[:],
                            wao[:, kh * 768 + m * 128:kh * 768 + m * 128 + 128],
                            oT[:, kh * R + c * RC:kh * R + (c + 1) * RC],
                            start=(kh == 0), stop=(kh == 7))
                    ta = p3s.tile([128, RC], BF16)
                    nc.scalar.activation(out=ta[:], in_=ps[:], func=AF.Identity,
                                         bias=small["bao"][:, m:m + 1], scale=1.0)
                    tg = small["tgt0"][:, m * 100:(m + 1) * 100]
                    nc.vector.tensor_tensor(out=t2[:, m * RC:(m + 1) * RC],
                                            in0=ta[:],
                                            in1=_ap(tg, [[0, 4], [1, 100]]),
                                            op=ALU.add)
                y2 = p3t.tile([128, 6 * RC], BF16)
                if ffn_triv:
                    # b1=b2=0 and trivial LN gains: relu is positive-
                    # homogeneous and LN3 is row-scale invariant, so LN2's
                    # rstd can be dropped entirely; center by mean only.
                    s1 = p3st.tile([1, RC], F32)
                    for k in range(6):
                        nc.tensor.matmul(s1[:], ones_col[:],
                                         t2[:, k * RC:(k + 1) * RC],
                                         start=(k == 0), stop=(k == 5))
                    nmean_b = p3f.tile([1, RC], BF16)
                    nc.vector.tensor_scalar_mul(out=nmean_b[:], in0=s1[:],
                                                scalar1=-1.0 / D)
                    pb = p3ab.tile([128, RC], F32)
                    nc.tensor.matmul(pb[:], ones_row[:], nmean_b[:],
                                     start=True, stop=True)
                    for k in range(6):
                        nc.vector.tensor_tensor(
                            out=y2[:, k * RC:(k + 1) * RC],
                            in0=t2[:, k * RC:(k + 1) * RC],
                            in1=pb[:], op=ALU.add)
                else:
                    layer_norm_T(t2, "ln2g", "ln2b",
                                 lambda k: y2[:, k * RC:(k + 1) * RC])
                ff1 = p3f.tile([128, 16 * RC], BF16)
                for mf in range(16):
                    ps = p3ps.tile([128, RC], F32)
                    for k in range(6):
                        nc.tensor.matmul(
                            ps[:],
                            w1[:, k * 2048 + mf * 128:k * 2048 + mf * 128 + 128],
                            y2[:, k * RC:(k + 1) * RC],
                            start=(k == 0), stop=(k == 5))
                    nc.scalar.activation(out=ff1[:, mf * RC:(mf + 1) * RC],
                                         in_=ps[:], func=AF.Relu,
                                         bias=small["b1"][:, mf:mf + 1],
                                         scale=1.0)
                t3 = p3t.tile([128, 6 * RC], BF16)
                for m in range(6):
                    ps = p3ps.tile([128, RC], F32)
                    for k in range(16):
                        nc.tensor.matmul(
                            ps[:],
                            w2[:, k * 768 + m * 128:k * 768 + m * 128 + 128],
                            ff1[:, k * RC:(k + 1) * RC],
                            start=(k == 0), stop=(k == 15))
                    tb = p3s.tile([128, RC], BF16)
                    nc.scalar.activation(out=tb[:], in_=ps[:], func=AF.Identity,
                                         bias=small["b2"][:, m:m + 1], scale=1.0)
                    nc.vector.tensor_tensor(out=t3[:, m * RC:(m + 1) * RC],
                                            in0=tb[:],
                                            in1=y2[:, m * RC:(m + 1) * RC],
                                            op=ALU.add)
                layer_norm_T(t3, "ln3g", "ln3b",
                             lambda k: h3T[:, k * R + c * RC:k * R + (c + 1) * RC])

        # ---- P4: GroupFC -> logitsT ----
        with pool("p4d", bufs=12) as p4d, pool("p4o") as p4o, \
             pool("p4ps", bufs=2, space="PSUM") as p4ps:
            logitsT = p4o.tile([96, G * BL], F32)
            dupb = p4o.tile(list(d["dupb"].shape), BF16)
            nc.sync.dma_start(out=dupb, in_=d["dupb"])
            for g0 in range(0, G, 16):
                ng = min(16, G - g0)
                ps = p4ps.tile([96, 16 * BL], F32)
                for gi in range(ng):
                    g = g0 + gi
                    dup = p4d.tile([128, 6 * 96], BF16)
                    nc.sync.dma_start(out=dup, in_=d["dup"][g])
                    if not skip_dupb:
                        nc.tensor.matmul(ps[:, gi * BL:(gi + 1) * BL],
                                         dupb[:, g * 96:(g + 1) * 96],
                                         ones32[:], start=True, stop=False)
                    for k in range(6):
                        hsl = _ap(h3T[:, k * R + g:], [[100, BL]])
                        nc.tensor.matmul(ps[:, gi * BL:(gi + 1) * BL],
                                         dup[:, k * 96:(k + 1) * 96],
                                         hsl, start=(skip_dupb and k == 0),
                                         stop=(k == 5))
                nc.vector.tensor_copy(out=logitsT[:, g0 * BL:(g0 + ng) * BL],
                                      in_=ps[:, 0:ng * BL])
            nc.sync.dma_start(out=out_d, in_=logitsT[:])


_CACHE = {}


def kernel(**inputs):
    f32 = lambda k: np.asarray(inputs[k], np.float32)
    x = f32("x")
    w_qkv, b_qkv = f32("w_qkv"), f32("b_qkv")
    w_attn_out, b_attn_out = f32("w_attn_out"), f32("b_attn_out")

    # host constant folding for the batch-independent query path
    t = 2.0 * f32("query_embed")
    mu = t.mean(-1, keepdims=True)
    va = ((t - mu) ** 2).mean(-1, keepdims=True)
    tgt0 = (t - mu) / np.sqrt(va + EPS) * f32("ln1_g") + f32("ln1_b")
    q = (tgt0 @ w_qkv[:, :D] + b_qkv[:D]) / np.sqrt(float(HD))
    bk = b_qkv[D:2 * D]
    qbk = np.stack([q[:, h * HD:(h + 1) * HD] @ bk[h * HD:(h + 1) * HD]
                    for h in range(H)], axis=1)
    bv = b_qkv[2 * D:]
    bao_eff = b_attn_out + bv @ w_attn_out   # softmax rows sum to 1

    col6 = lambda a: np.ascontiguousarray(a.reshape(6, 128).T)
    feed = {
        "wemb": _bf(f32("w_embed").reshape(16, 128, 768).transpose(1, 0, 2)
                    .reshape(128, -1)),
        "be": col6(f32("b_embed")),
        "wk": _bf(w_qkv[:, D:2 * D].reshape(6, 128, 768).transpose(1, 0, 2)
                  .reshape(128, -1)),
        "wv": _bf(w_qkv[:, 2 * D:].reshape(6, 128, 768).transpose(1, 0, 2)
                  .reshape(128, -1)),
        "wao": _bf(w_attn_out.reshape(8, 96, 768).transpose(1, 0, 2)
                   .reshape(96, -1)),
        "bao": col6(bao_eff),
        "w1": _bf(f32("w1").reshape(6, 128, 2048).transpose(1, 0, 2)
                  .reshape(128, -1)),
        "b1": np.ascontiguousarray(f32("b1").reshape(16, 128).T),
        "w2": _bf(f32("w2").reshape(16, 128, 768).transpose(1, 0, 2)
                  .reshape(128, -1)),
        "b2": col6(f32("b2")),
        "qT": _bf(q.T.reshape(8, 96, 100).transpose(1, 0, 2).reshape(96, -1)),
        "qbk": np.ascontiguousarray(qbk.astype(np.float32)),
        "tgt0": _bf(tgt0.T.reshape(6, 128, 100).transpose(1, 0, 2)
                    .reshape(128, -1)),
        "ln2g": col6(f32("ln2_g")), "ln2b": col6(f32("ln2_b")),
        "ln3g": col6(f32("ln3_g")), "ln3b": col6(f32("ln3_b")),
        "dup": _bf(f32("dup_pool").reshape(G, 6, 128, 96).transpose(0, 2, 1, 3)
                   .reshape(G, 128, 6 * 96)),
        "dupb": _bf(f32("dup_bias").reshape(1, -1)),
    }

    skip_dupb = bool(np.all(f32("dup_bias") == 0.0))
    ln_triv = bool(np.all(f32("ln2_g") == 1.0) and np.all(f32("ln2_b") == 0.0)
                   and np.all(f32("ln3_g") == 1.0) and np.all(f32("ln3_b") == 0.0))
    ffn_triv = bool(ln_triv and np.all(f32("b1") == 0.0)
                    and np.all(f32("b2") == 0.0))
    key = ("nc", skip_dupb, ln_triv, ffn_triv)
    if key not in _CACHE:
        _CACHE[key] = build_program(skip_dupb, ln_triv, ffn_triv)
    nc = _CACHE[key]
    _CACHE["nc"] = nc

    # xr[core] axes: [c, col, k, p]; device wants [c, p, k, col]
    xr = x.reshape(NCORES, XCH, XCOLS, 16, 128)
    in_maps = []
    for core in range(NCORES):
        xT = xr[core].transpose(0, 3, 2, 1).reshape(XCH, 128, 16 * XCOLS)
        in_maps.append({**feed, "xT": _bf(xT)})

    _CACHE["in_maps"] = in_maps
    res = run_bass_kernel_spmd(nc, in_maps, list(range(NCORES)))
    outs = []
    for core in range(NCORES):
        lt = np.asarray(res.results[core]["logitsT"], np.float32)
        outs.append(lt.reshape(96, G, BL).transpose(2, 1, 0).reshape(BL, G * DF))
    return np.concatenate(outs, axis=0).astype(np.float32)


# revision 13
# speedup vs baseline: 1.1957x; 1.0037x over previous
"""Trainium2 Bass kernel for nn_MLDecoder (moe_routing).

Data-parallel over batch across 8 NeuronCores (32 batch rows/core, head params
replicated). Activations stay feature-major ("transposed"): C^T = W^T A^T via
matmul(out=C^T, lhsT=W(natural), rhs=A^T). Rows r = b*100+g (b-major). The
batch-independent query path (tgt0, q) is constant-folded on the host. All
matmuls bf16 with fp32 PSUM; LN stats via ones-matmuls; softmax without
max-subtraction (scores are O(1) for this head).
"""
import sys
sys.path.insert(0, "/opt/trn_rl_repo")

import numpy as np
import ml_dtypes

import concourse.bass as bass
from concourse import bacc
import concourse.tile as tile
import concourse.mybir as mybir
from concourse.bass import AP
from concourse.bass_utils import run_bass_kernel_spmd
from concourse.masks import make_identity

F32 = mybir.dt.float32
BF16 = mybir.dt.bfloat16
BF = ml_dtypes.bfloat16
AF = mybir.ActivationFunctionType
ALU = mybir.AluOpType
AX = mybir.AxisListType

B, S, C0 = 256, 49, 2048
D, F = 768, 2048
G, DF = 100, 96
H, HD = 8, 96
EPS = 1e-5
NCORES = 8
BL = B // NCORES          # 32 batch rows per core
R = BL * G                # 3200 rows (b,g) per core
RC = 400                  # row chunk = 4 b
NCHUNK = R // RC
XCH = 4                   # x col chunks (8 b each)
XCOLS = (BL // XCH) * S   # 392
PADS = 64                 # padded spatial stride
MCOLS = BL * PADS         # 2048 padded mem cols


def _bf(a):
    return np.ascontiguousarray(a.astype(BF))


def _ap(base, free_dims):
    """Replace the free dims of a (sliced) AP, keeping its partition dim."""
    return AP(tensor=base.tensor, offset=base.offset,
              ap=[base.ap[0]] + [list(fd) for fd in free_dims])


def build_program(skip_dupb=False, ln_triv=False, ffn_triv=False):
    nc = bacc.Bacc("TRN2", target_bir_lowering=False, debug=False,
                   num_devices=NCORES)
    d = {}

    def din(name, shape, dt):
        d[name] = nc.dram_tensor(name, list(shape), dt, kind="ExternalInput").ap()

    din("xT", (XCH, 128, 16 * XCOLS), BF16)
    din("wemb", (128, 16 * 768), BF16)
    din("be", (128, 6), F32)
    din("wk", (128, 6 * 768), BF16)
    din("wv", (128, 6 * 768), BF16)
    din("wao", (96, 8 * 768), BF16)
    din("bao", (128, 6), F32)
    din("w1", (128, 6 * 2048), BF16)
    din("b1", (128, 16), F32)
    din("w2", (128, 16 * 768), BF16)
    din("b2", (128, 6), F32)
    din("qT", (96, 8 * 100), BF16)
    din("qbk", (100, 8), F32)
    din("tgt0", (128, 6 * 100), BF16)
    din("ln2g", (128, 6), F32)
    din("ln2b", (128, 6), F32)
    din("ln3g", (128, 6), F32)
    din("ln3b", (128, 6), F32)
    din("dup", (100, 128, 6 * 96), BF16)
    din("dupb", (1, G * 96), BF16)
    out_d = nc.dram_tensor("logitsT", [96, G * BL], F32,
                           kind="ExternalOutput").ap()

    with tile.TileContext(nc) as tc:
        build_kernel(tc, d, out_d, skip_dupb, ln_triv, ffn_triv)
    nc.compile()
    return nc


def build_kernel(tc, d, out_d, skip_dupb=False, ln_triv=False, ffn_triv=False):
    nc = tc.nc

    def pool(name, bufs=1, space="SBUF"):
        return tc.tile_pool(name=name, bufs=bufs, space=space)

    with pool("resident") as res, pool("h3pool") as h3p, pool("oTpool") as oTp:
        ident = res.tile([128, 128], BF16)
        make_identity(nc, ident[:])
        ones_col = res.tile([128, 1], BF16)
        nc.vector.memset(ones_col[:], 1.0)
        ones_row = res.tile([1, 128], BF16)
        nc.vector.memset(ones_row[:], 1.0)
        ones32 = res.tile([1, BL], BF16)
        nc.vector.memset(ones32[:], 1.0)
        eps_t = res.tile([1, 1], F32)
        nc.vector.memset(eps_t[:], EPS)

        small = {}
        for name in ["be", "bao", "b1", "b2", "qT", "qbk", "tgt0",
                     "ln2g", "ln2b", "ln3g", "ln3b"]:
            t = res.tile(list(d[name].shape), d[name].dtype, tag=name)
            nc.sync.dma_start(out=t, in_=d[name])
            small[name] = t

        h3T = h3p.tile([128, 6 * R], BF16)
        oT = oTp.tile([96, 8 * R], BF16)

        with pool("memTpool") as memp:
            memT = memp.tile([128, 6 * MCOLS], BF16)

            # ---- P0: mem^T = relu(We^T x^T + be), written b-padded ----
            with pool("p0w") as p0w, pool("p0x", bufs=2) as p0x, \
                 pool("p0ps", bufs=3, space="PSUM") as p0ps:
                wemb = p0w.tile([128, 16 * 768], BF16)
                nc.sync.dma_start(out=wemb, in_=d["wemb"])
                for c in range(XCH):
                    xt = p0x.tile([128, 16 * XCOLS], BF16)
                    nc.sync.dma_start(out=xt, in_=d["xT"][c])
                    for m in range(6):
                        ps = p0ps.tile([128, XCOLS], F32)
                        for k in range(16):
                            nc.tensor.matmul(
                                ps[:],
                                wemb[:, k * 768 + m * 128:k * 768 + m * 128 + 128],
                                xt[:, k * XCOLS:(k + 1) * XCOLS],
                                start=(k == 0), stop=(k == 15))
                        dst = _ap(memT[:, m * MCOLS + c * 8 * PADS:],
                                  [[PADS, 8], [1, S]])
                        src = _ap(ps[:], [[S, 8], [1, S]])
                        nc.scalar.activation(out=dst, in_=src, func=AF.Relu,
                                             bias=small["be"][:, m:m + 1],
                                             scale=1.0)

            # ---- P1: K^T (head-major, b-padded) and V (rows padded) ----
            with pool("kvpool") as kvp:
                KT = kvp.tile([96, 8 * MCOLS], BF16)
                Vp = kvp.tile([128, 16 * 768], BF16)
                with pool("p1w") as p1w, \
                     pool("p1ps", bufs=3, space="PSUM") as p1ps:
                    wk = p1w.tile([128, 6 * 768], BF16)
                    nc.sync.dma_start(out=wk, in_=d["wk"])
                    wv = p1w.tile([128, 6 * 768], BF16)
                    nc.sync.dma_start(out=wv, in_=d["wv"])
                    for h in range(H):
                        for c in range(XCH):
                            ps = p1ps.tile([96, XCOLS], F32)
                            for k in range(6):
                                rhs = _ap(memT[:, k * MCOLS + c * 8 * PADS:],
                                          [[PADS, 8], [1, S]])
                                nc.tensor.matmul(
                                    ps[:],
                                    wk[:, k * 768 + h * 96:k * 768 + h * 96 + 96],
                                    rhs, start=(k == 0), stop=(k == 5))
                            dst = _ap(KT[:, h * MCOLS + c * 8 * PADS:],
                                      [[PADS, 8], [1, S]])
                            nc.vector.tensor_copy(
                                out=dst, in_=_ap(ps[:], [[S, 8], [1, S]]))
                    for t in range(16):
                        ps = p1ps.tile([128, 768], F32)
                        for sub in range(2):
                            n0, n1 = sub * 512, min(768, (sub + 1) * 512)
                            for k in range(6):
                                nc.tensor.matmul(
                                    ps[:, n0:n1],
                                    memT[:, k * MCOLS + t * 128:
                                         k * MCOLS + t * 128 + 128],
                                    wv[:, k * 768 + n0:k * 768 + n1],
                                    start=(k == 0), stop=(k == 5))
                        nc.vector.tensor_copy(out=Vp[:, t * 768:(t + 1) * 768],
                                              in_=ps[:])

                # ---- P2: attention ----
                with pool("p2a", bufs=2) as p2a, pool("p2s", bufs=3) as p2s, \
                     pool("p2ps", bufs=2, space="PSUM") as psc, \
                     pool("p2pt", bufs=2, space="PSUM") as pst, \
                     pool("p2po", bufs=2, space="PSUM") as pso:
                    for bg in range(4):
                        attnT = p2a.tile([128, 8 * 400], BF16)
                        for h in range(H):
                            ps = psc.tile([100, 8 * S], F32)
                            rhs = _ap(KT[:, h * MCOLS + bg * 8 * PADS:],
                                      [[PADS, 8], [1, S]])
                            nc.tensor.matmul(ps[:],
                                             small["qT"][:, h * 100:(h + 1) * 100],
                                             rhs, start=True, stop=True)
                            # exp into 64-padded slots (pads hold garbage,
                            # excluded by every later access pattern)
                            att = p2s.tile([100, 8 * PADS], BF16)
                            nc.scalar.activation(out=_ap(att[:], [[PADS, 8], [1, S]]),
                                                 in_=ps[:],
                                                 func=AF.Exp,
                                                 bias=small["qbk"][:, h:h + 1],
                                                 scale=1.0)
                            sums = p2s.tile([100, 8], F32)
                            nc.vector.reduce_sum(out=sums[:],
                                                 in_=_ap(att[:], [[PADS, 8], [1, S]]),
                                                 axis=AX.X)
                            inv = p2s.tile([100, 8], F32)
                            nc.vector.reciprocal(out=inv[:], in_=sums[:])
                            attn = p2s.tile([100, 8 * PADS], BF16)
                            nc.vector.tensor_tensor(
                                out=_ap(attn[:], [[PADS, 8], [1, S]]),
                                in0=_ap(att[:], [[PADS, 8], [1, S]]),
                                in1=_ap(inv[:], [[1, 8], [0, S]]),
                                op=ALU.mult)
                            for pr in range(4):
                                pt = pst.tile([128, 100], BF16)
                                nc.tensor.transpose(
                                    pt[:], attn[:, pr * 128:(pr + 1) * 128],
                                    ident[0:100, 0:100])
                                nc.vector.tensor_copy(
                                    out=attnT[:, h * 400 + pr * 100:
                                              h * 400 + pr * 100 + 100],
                                    in_=pt[:])
                        for lb in range(8):
                            b = bg * 8 + lb
                            po = pso.tile([96, 1024], F32)
                            for h in range(H):
                                vsl = Vp[(lb % 2) * 64:(lb % 2) * 64 + S,
                                         (b // 2) * 768 + h * 96:
                                         (b // 2) * 768 + h * 96 + 96]
                                nc.tensor.matmul(
                                    po[:, h * 128:h * 128 + 100], vsl,
                                    attnT[(lb % 2) * 64:(lb % 2) * 64 + S,
                                          h * 400 + (lb // 2) * 100:
                                          h * 400 + (lb // 2) * 100 + 100],
                                    start=True, stop=True)
                            dst = _ap(oT[:, b * 100:], [[R, 8], [1, 100]])
                            nc.vector.tensor_copy(
                                out=dst, in_=_ap(po[:], [[128, 8], [1, 100]]))

        # ---- P3: attn_out + LN2 + FFN + LN3 -> h3T ----
        with pool("p3w") as p3w, pool("p3t") as p3t, \
             pool("p3f") as p3f, pool("p3s", bufs=2) as p3s, \
             pool("p3ps", bufs=4, space="PSUM") as p3ps, \
             pool("p3st", space="PSUM") as p3st, \
             pool("p3ab", space="PSUM") as p3ab:
            wao = p3w.tile([96, 8 * 768], BF16)
            nc.sync.dma_start(out=wao, in_=d["wao"])
            w1 = p3w.tile([128, 6 * 2048], BF16)
            nc.sync.dma_start(out=w1, in_=d["w1"])
            w2 = p3w.tile([128, 16 * 768], BF16)
            nc.sync.dma_start(out=w2, in_=d["w2"])

            def layer_norm_T(xin, gname, bname, yout):
                sq = p3f.tile([128, 6 * RC], BF16)
                nc.scalar.square(out=sq[:], in_=xin[:])
                s1 = p3st.tile([1, RC], F32)
                s2 = p3st.tile([1, RC], F32)
                for k in range(6):
                    nc.tensor.matmul(s1[:], ones_col[:],
                                     xin[:, k * RC:(k + 1) * RC],
                                     start=(k == 0), stop=(k == 5))
                for k in range(6):
                    nc.tensor.matmul(s2[:], ones_col[:],
                                     sq[:, k * RC:(k + 1) * RC],
                                     start=(k == 0), stop=(k == 5))
                mean = p3f.tile([1, RC], F32)
                nc.vector.tensor_scalar_mul(out=mean[:], in0=s1[:],
                                            scalar1=1.0 / D)
                var = p3f.tile([1, RC], F32)
                nc.vector.tensor_scalar_mul(out=var[:], in0=s2[:],
                                            scalar1=1.0 / D)
                msq = p3f.tile([1, RC], F32)
                nc.vector.tensor_tensor(out=msq[:], in0=mean[:], in1=mean[:],
                                        op=ALU.mult)
                nc.vector.tensor_tensor(out=var[:], in0=var[:], in1=msq[:],
                                        op=ALU.subtract)
                sd = p3f.tile([1, RC], F32)
                nc.scalar.activation(out=sd[:], in_=var[:], func=AF.Sqrt,
                                     bias=eps_t[:], scale=1.0)
                rstd = p3f.tile([1, RC], F32)
                nc.vector.reciprocal(out=rstd[:], in_=sd[:])
                nmr = p3f.tile([1, RC], F32)
                nc.vector.tensor_tensor(out=nmr[:], in0=mean[:], in1=rstd[:],
                                        op=ALU.mult)
                rstd_b = p3f.tile([1, RC], BF16)
                nc.vector.tensor_copy(out=rstd_b[:], in_=rstd[:])
                nmr_b = p3f.tile([1, RC], BF16)
                nc.vector.tensor_scalar_mul(out=nmr_b[:], in0=nmr[:], scalar1=-1.0)
                pa = p3ab.tile([128, RC], F32)
                nc.tensor.matmul(pa[:], ones_row[:], rstd_b[:],
                                 start=True, stop=True)
                pb = p3ab.tile([128, RC], F32)
                nc.tensor.matmul(pb[:], ones_row[:], nmr_b[:],
                                 start=True, stop=True)
                gv, bv = small[gname], small[bname]
                for k in range(6):
                    u = p3s.tile([128, RC], F32)
                    nc.vector.tensor_tensor(out=u[:],
                                            in0=xin[:, k * RC:(k + 1) * RC],
                                            in1=pa[:], op=ALU.mult)
                    if ln_triv:
                        nc.vector.tensor_tensor(out=yout(k), in0=u[:],
                                                in1=pb[:], op=ALU.add)
                    else:
                        nc.vector.tensor_tensor(out=u[:], in0=u[:], in1=pb[:],
                                                op=ALU.add)
                        nc.vector.tensor_scalar(out=yout(k), in0=u[:],
                                                scalar1=gv[:, k:k + 1],
                                                scalar2=bv[:, k:k + 1],
                                                op0=ALU.mult, op1=ALU.add)

            for c in range(NCHUNK):
                t2 = p3t.tile([128, 6 * RC], BF16)
                for m in range(6):
                    ps = p3ps.tile([128, RC], F32)
                    for kh in range(H):
                        nc.tensor.matmul(
                            ps[:],
                            wao[:, kh * 768 + m * 128:kh * 768 + m * 128 + 128],
                            oT[:, kh * R + c * RC:kh * R + (c + 1) * RC],
                            start=(kh == 0), stop=(kh == 7))
                    ta = p3s.tile([128, RC], BF16)
                    nc.scalar.activation(out=ta[:], in_=ps[:], func=AF.Identity,
                                         bias=small["bao"][:, m:m + 1], scale=1.0)
                    tg = small["tgt0"][:, m * 100:(m + 1) * 100]
                    nc.vector.tensor_tensor(out=t2[:, m * RC:(m + 1) * RC],
                                            in0=ta[:],
                                            in1=_ap(tg, [[0, 4], [1, 100]]),
                                            op=ALU.add)
                y2 = p3t.tile([128, 6 * RC], BF16)
                if ffn_triv:
                    # b1=b2=0 and trivial LN gains: relu is positive-
                    # homogeneous and LN3 is row-scale invariant, so LN2's
                    # rstd can be dropped entirely; center by mean only.
                    s1 = p3st.tile([1, RC], F32)
                    for k in range(6):
                        nc.tensor.matmul(s1[:], ones_col[:],
                                         t2[:, k * RC:(k + 1) * RC],
                                         start=(k == 0), stop=(k == 5))
                    nmean_b = p3f.tile([1, RC], BF16)
                    nc.vector.tensor_scalar_mul(out=nmean_b[:], in0=s1[:],
                                                scalar1=-1.0 / D)
                    pb = p3ab.tile([128, RC], F32)
                    nc.tensor.matmul(pb[:], ones_row[:], nmean_b[:],
                                     start=True, stop=True)
                    for k in range(6):
                        nc.vector.tensor_tensor(
                            out=y2[:, k * RC:(k + 1) * RC],
                            in0=t2[:, k * RC:(k + 1) * RC],
                            in1=pb[:], op=ALU.add)
                else:
                    layer_norm_T(t2, "ln2g", "ln2b",
                                 lambda k: y2[:, k * RC:(k + 1) * RC])
                ff1 = p3f.tile([128, 16 * RC], BF16)
                for mf in range(16):
                    ps = p3ps.tile([128, RC], F32)
                    for k in range(6):
                        nc.tensor.matmul(
                            ps[:],
                            w1[:, k * 2048 + mf * 128:k * 2048 + mf * 128 + 128],
                            y2[:, k * RC:(k + 1) * RC],
                            start=(k == 0), stop=(k == 5))
                    nc.scalar.activation(out=ff1[:, mf * RC:(mf + 1) * RC],
                                         in_=ps[:], func=AF.Relu,
                                         bias=small["b1"][:, mf:mf + 1],
                                         scale=1.0)
                t3 = p3t.tile([128, 6 * RC], BF16)
                for m in range(6):
                    ps = p3ps.tile([128, RC], F32)
                    for k in range(16):
                        nc.tensor.matmul(
                            ps[:],
                            w2[:, k * 768 + m * 128:k * 768 + m * 128 + 128],
                            ff1[:, k * RC:(k + 1) * RC],
                            start=(k == 0), stop=(k == 15))
                    tb = p3s.tile([128, RC], BF16)
                    nc.scalar.activation(out=tb[:], in_=ps[:], func=AF.Identity,
                                         bias=small["b2"][:, m:m + 1], scale=1.0)
                    nc.vector.tensor_tensor(out=t3[:, m * RC:(m + 1) * RC],
                                            in0=tb[:],
                                            in1=y2[:, m * RC:(m + 1) * RC],
                                            op=ALU.add)
                layer_norm_T(t3, "ln3g", "ln3b",
                             lambda k: h3T[:, k * R + c * RC:k * R + (c + 1) * RC])

        # ---- P4: GroupFC -> logitsT ----
        with pool("p4d", bufs=16) as p4d, pool("p4o") as p4o, \
             pool("p4ps", bufs=2, space="PSUM") as p4ps:
            logitsT = p4o.tile([96, G * BL], F32)
            dupb = p4o.tile(list(d["dupb"].shape), BF16)
            nc.sync.dma_start(out=dupb, in_=d["dupb"])
            for g0 in range(0, G, 16):
                ng = min(16, G - g0)
                ps = p4ps.tile([96, 16 * BL], F32)
                for gi in range(ng):
                    g = g0 + gi
                    dup = p4d.tile([128, 6 * 96], BF16)
                    nc.sync.dma_start(out=dup, in_=d["dup"][g])
                    if not skip_dupb:
                        nc.tensor.matmul(ps[:, gi * BL:(gi + 1) * BL],
                                         dupb[:, g * 96:(g + 1) * 96],
                                         ones32[:], start=True, stop=False)
                    for k in range(6):
                        hsl = _ap(h3T[:, k * R + g:], [[100, BL]])
                        nc.tensor.matmul(ps[:, gi * BL:(gi + 1) * BL],
                                         dup[:, k * 96:(k + 1) * 96],
                                         hsl, start=(skip_dupb and k == 0),
                                         stop=(k == 5))
                nc.vector.tensor_copy(out=logitsT[:, g0 * BL:(g0 + ng) * BL],
                                      in_=ps[:, 0:ng * BL])
            nc.sync.dma_start(out=out_d, in_=logitsT[:])


_CACHE = {}


def kernel(**inputs):
    f32 = lambda k: np.asarray(inputs[k], np.float32)
    x = f32("x")
    w_qkv, b_qkv = f32("w_qkv"), f32("b_qkv")
    w_attn_out, b_attn_out = f32("w_attn_out"), f32("b_attn_out")

    # host constant folding for the batch-independent query path
    t = 2.0 * f32("query_embed")
    mu = t.mean(-1, keepdims=True)
    va = ((t - mu) ** 2).mean(-1, keepdims=True)
    tgt0 = (t - mu) / np.sqrt(va + EPS) * f32("ln1_g") + f32("ln1_b")
    q = (tgt0 @ w_qkv[:, :D] + b_qkv[:D]) / np.sqrt(float(HD))
    bk = b_qkv[D:2 * D]
    qbk = np.stack([q[:, h * HD:(h + 1) * HD] @ bk[h * HD:(h + 1) * HD]
                    for h in range(H)], axis=1)
    bv = b_qkv[2 * D:]
    bao_eff = b_attn_out + bv @ w_attn_out   # softmax rows sum to 1

    col6 = lambda a: np.ascontiguousarray(a.reshape(6, 128).T)
    feed = {
        "wemb": _bf(f32("w_embed").reshape(16, 128, 768).transpose(1, 0, 2)
                    .reshape(128, -1)),
        "be": col6(f32("b_embed")),
        "wk": _bf(w_qkv[:, D:2 * D].reshape(6, 128, 768).transpose(1, 0, 2)
                  .reshape(128, -1)),
        "wv": _bf(w_qkv[:, 2 * D:].reshape(6, 128, 768).transpose(1, 0, 2)
                  .reshape(128, -1)),
        "wao": _bf(w_attn_out.reshape(8, 96, 768).transpose(1, 0, 2)
                   .reshape(96, -1)),
        "bao": col6(bao_eff),
        "w1": _bf(f32("w1").reshape(6, 128, 2048).transpose(1, 0, 2)
                  .reshape(128, -1)),
        "b1": np.ascontiguousarray(f32("b1").reshape(16, 128).T),
        "w2": _bf(f32("w2").reshape(16, 128, 768).transpose(1, 0, 2)
                  .reshape(128, -1)),
        "b2": col6(f32("b2")),
        "qT": _bf(q.T.reshape(8, 96, 100).transpose(1, 0, 2).reshape(96, -1)),
        "qbk": np.ascontiguousarray(qbk.astype(np.float32)),
        "tgt0": _bf(tgt0.T.reshape(6, 128, 100).transpose(1, 0, 2)
                    .reshape(128, -1)),
        "ln2g": col6(f32("ln2_g")), "ln2b": col6(f32("ln2_b")),
        "ln3g": col6(f32("ln3_g")), "ln3b": col6(f32("ln3_b")),
        "dup": _bf(f32("dup_pool").reshape(G, 6, 128, 96).transpose(0, 2, 1, 3)
                   .reshape(G, 128, 6 * 96)),
        "dupb": _bf(f32("dup_bias").reshape(1, -1)),
    }

    skip_dupb = bool(np.all(f32("dup_bias") == 0.0))
    ln_triv = bool(np.all(f32("ln2_g") == 1.0) and np.all(f32("ln2_b") == 0.0)
                   and np.all(f32("ln3_g") == 1.0) and np.all(f32("ln3_b") == 0.0))
    ffn_triv = bool(ln_triv and np.all(f32("b1") == 0.0)
                    and np.all(f32("b2") == 0.0))
    key = ("nc", skip_dupb, ln_triv, ffn_triv)
    if key not in _CACHE:
        _CACHE[key] = build_program(skip_dupb, ln_triv, ffn_triv)
    nc = _CACHE[key]
    _CACHE["nc"] = nc

    # xr[core] axes: [c, col, k, p]; device wants [c, p, k, col]
    xr = x.reshape(NCORES, XCH, XCOLS, 16, 128)
    in_maps = []
    for core in range(NCORES):
        xT = xr[core].transpose(0, 3, 2, 1).reshape(XCH, 128, 16 * XCOLS)
        in_maps.append({**feed, "xT": _bf(xT)})

    _CACHE["in_maps"] = in_maps
    res = run_bass_kernel_spmd(nc, in_maps, list(range(NCORES)))
    outs = []
    for core in range(NCORES):
        lt = np.asarray(res.results[core]["logitsT"], np.float32)
        outs.append(lt.reshape(96, G, BL).transpose(2, 1, 0).reshape(BL, G * DF))
    return np.concatenate(outs, axis=0).astype(np.float32)


# revision 14
# speedup vs baseline: 1.2756x; 1.0668x over previous
"""Trainium2 Bass kernel for nn_MLDecoder (moe_routing).

Data-parallel over batch across 8 NeuronCores (32 batch rows/core, head params
replicated). Activations stay feature-major ("transposed"): C^T = W^T A^T via
matmul(out=C^T, lhsT=W(natural), rhs=A^T). Rows r = b*100+g (b-major). The
batch-independent query path (tgt0, q) is constant-folded on the host. All
matmuls bf16 with fp32 PSUM; LN stats via ones-matmuls; softmax without
max-subtraction (scores are O(1) for this head).
"""
import sys
sys.path.insert(0, "/opt/trn_rl_repo")

import numpy as np
import ml_dtypes

import concourse.bass as bass
from concourse import bacc
import concourse.tile as tile
import concourse.mybir as mybir
from concourse.bass import AP
from concourse.bass_utils import run_bass_kernel_spmd
from concourse.masks import make_identity

F32 = mybir.dt.float32
BF16 = mybir.dt.bfloat16
BF = ml_dtypes.bfloat16
AF = mybir.ActivationFunctionType
ALU = mybir.AluOpType
AX = mybir.AxisListType

B, S, C0 = 256, 49, 2048
D, F = 768, 2048
G, DF = 100, 96
H, HD = 8, 96
EPS = 1e-5
NCORES = 8
BL = B // NCORES          # 32 batch rows per core
R = BL * G                # 3200 rows (b,g) per core
RC = 400                  # row chunk = 4 b
NCHUNK = R // RC
XCH = 4                   # x col chunks (8 b each)
XCOLS = (BL // XCH) * S   # 392
PADS = 64                 # padded spatial stride
MCOLS = BL * PADS         # 2048 padded mem cols


def _bf(a):
    return np.ascontiguousarray(a.astype(BF))


def _ap(base, free_dims):
    """Replace the free dims of a (sliced) AP, keeping its partition dim."""
    return AP(tensor=base.tensor, offset=base.offset,
              ap=[base.ap[0]] + [list(fd) for fd in free_dims])


def build_program(skip_dupb=False, ln_triv=False, ffn_triv=False):
    nc = bacc.Bacc("TRN2", target_bir_lowering=False, debug=False,
                   num_devices=NCORES)
    d = {}

    def din(name, shape, dt):
        d[name] = nc.dram_tensor(name, list(shape), dt, kind="ExternalInput").ap()

    din("xT", (XCH, 128, 16 * XCOLS), BF16)
    din("wemb", (128, 16 * 768), BF16)
    din("be", (128, 6), F32)
    din("wk", (128, 6 * 768), BF16)
    din("wv", (128, 6 * 768), BF16)
    din("wao", (96, 8 * 768), BF16)
    din("bao", (128, 6), F32)
    din("w1", (128, 6 * 2048), BF16)
    din("b1", (128, 16), F32)
    din("w2", (128, 16 * 768), BF16)
    din("b2", (128, 6), F32)
    din("qT", (96, 8 * 100), BF16)
    din("qbk", (100, 8), F32)
    din("tgt0", (128, 6 * 100), BF16)
    din("ln2g", (128, 6), F32)
    din("ln2b", (128, 6), F32)
    din("ln3g", (128, 6), F32)
    din("ln3b", (128, 6), F32)
    din("dup", (100, 128, 6 * 96), BF16)
    din("dupb", (1, G * 96), BF16)
    out_d = nc.dram_tensor("logitsT", [96, G * BL], F32,
                           kind="ExternalOutput").ap()

    with tile.TileContext(nc) as tc:
        build_kernel(tc, d, out_d, skip_dupb, ln_triv, ffn_triv)
    nc.compile()
    return nc


def build_kernel(tc, d, out_d, skip_dupb=False, ln_triv=False, ffn_triv=False):
    nc = tc.nc

    def pool(name, bufs=1, space="SBUF"):
        return tc.tile_pool(name=name, bufs=bufs, space=space)

    with pool("resident") as res, pool("h3pool") as h3p, pool("oTpool") as oTp:
        ident = res.tile([128, 128], BF16)
        make_identity(nc, ident[:])
        ones_col = res.tile([128, 1], BF16)
        nc.vector.memset(ones_col[:], 1.0)
        ones_row = res.tile([1, 128], BF16)
        nc.vector.memset(ones_row[:], 1.0)
        ones32 = res.tile([1, BL], BF16)
        nc.vector.memset(ones32[:], 1.0)
        eps_t = res.tile([1, 1], F32)
        nc.vector.memset(eps_t[:], EPS)

        small = {}
        for name in ["be", "bao", "b1", "b2", "qT", "qbk", "tgt0",
                     "ln2g", "ln2b", "ln3g", "ln3b"]:
            t = res.tile(list(d[name].shape), d[name].dtype, tag=name)
            nc.sync.dma_start(out=t, in_=d[name])
            small[name] = t

        h3T = h3p.tile([128, 6 * R], BF16)
        rstd_all = h3p.tile([1, R], BF16)
        oT = oTp.tile([96, 8 * R], BF16)

        with pool("memTpool") as memp:
            memT = memp.tile([128, 6 * MCOLS], BF16)

            # ---- P0: mem^T = relu(We^T x^T + be), written b-padded ----
            with pool("p0w") as p0w, pool("p0x", bufs=2) as p0x, \
                 pool("p0ps", bufs=3, space="PSUM") as p0ps:
                wemb = p0w.tile([128, 16 * 768], BF16)
                nc.sync.dma_start(out=wemb, in_=d["wemb"])
                for c in range(XCH):
                    xt = p0x.tile([128, 16 * XCOLS], BF16)
                    nc.sync.dma_start(out=xt, in_=d["xT"][c])
                    for m in range(6):
                        ps = p0ps.tile([128, XCOLS], F32)
                        for k in range(16):
                            nc.tensor.matmul(
                                ps[:],
                                wemb[:, k * 768 + m * 128:k * 768 + m * 128 + 128],
                                xt[:, k * XCOLS:(k + 1) * XCOLS],
                                start=(k == 0), stop=(k == 15))
                        dst = _ap(memT[:, m * MCOLS + c * 8 * PADS:],
                                  [[PADS, 8], [1, S]])
                        src = _ap(ps[:], [[S, 8], [1, S]])
                        nc.scalar.activation(out=dst, in_=src, func=AF.Relu,
                                             bias=small["be"][:, m:m + 1],
                                             scale=1.0)

            # ---- P1: K^T (head-major, b-padded) and V (rows padded) ----
            with pool("kvpool") as kvp:
                KT = kvp.tile([96, 8 * MCOLS], BF16)
                Vp = kvp.tile([128, 16 * 768], BF16)
                with pool("p1w") as p1w, \
                     pool("p1ps", bufs=3, space="PSUM") as p1ps:
                    wk = p1w.tile([128, 6 * 768], BF16)
                    nc.sync.dma_start(out=wk, in_=d["wk"])
                    wv = p1w.tile([128, 6 * 768], BF16)
                    nc.sync.dma_start(out=wv, in_=d["wv"])
                    for h in range(H):
                        for c in range(XCH):
                            ps = p1ps.tile([96, XCOLS], F32)
                            for k in range(6):
                                rhs = _ap(memT[:, k * MCOLS + c * 8 * PADS:],
                                          [[PADS, 8], [1, S]])
                                nc.tensor.matmul(
                                    ps[:],
                                    wk[:, k * 768 + h * 96:k * 768 + h * 96 + 96],
                                    rhs, start=(k == 0), stop=(k == 5))
                            dst = _ap(KT[:, h * MCOLS + c * 8 * PADS:],
                                      [[PADS, 8], [1, S]])
                            nc.vector.tensor_copy(
                                out=dst, in_=_ap(ps[:], [[S, 8], [1, S]]))
                    for t in range(16):
                        ps = p1ps.tile([128, 768], F32)
                        for sub in range(2):
                            n0, n1 = sub * 512, min(768, (sub + 1) * 512)
                            for k in range(6):
                                nc.tensor.matmul(
                                    ps[:, n0:n1],
                                    memT[:, k * MCOLS + t * 128:
                                         k * MCOLS + t * 128 + 128],
                                    wv[:, k * 768 + n0:k * 768 + n1],
                                    start=(k == 0), stop=(k == 5))
                        nc.vector.tensor_copy(out=Vp[:, t * 768:(t + 1) * 768],
                                              in_=ps[:])

                # ---- P2: attention ----
                with pool("p2a", bufs=2) as p2a, pool("p2s", bufs=3) as p2s, \
                     pool("p2ps", bufs=2, space="PSUM") as psc, \
                     pool("p2pt", bufs=2, space="PSUM") as pst, \
                     pool("p2po", bufs=2, space="PSUM") as pso:
                    for bg in range(4):
                        attnT = p2a.tile([128, 8 * 400], BF16)
                        for h in range(H):
                            ps = psc.tile([100, 8 * S], F32)
                            rhs = _ap(KT[:, h * MCOLS + bg * 8 * PADS:],
                                      [[PADS, 8], [1, S]])
                            nc.tensor.matmul(ps[:],
                                             small["qT"][:, h * 100:(h + 1) * 100],
                                             rhs, start=True, stop=True)
                            # exp into 64-padded slots (pads hold garbage,
                            # excluded by every later access pattern)
                            att = p2s.tile([100, 8 * PADS], BF16)
                            nc.scalar.activation(out=_ap(att[:], [[PADS, 8], [1, S]]),
                                                 in_=ps[:],
                                                 func=AF.Exp,
                                                 bias=small["qbk"][:, h:h + 1],
                                                 scale=1.0)
                            sums = p2s.tile([100, 8], F32)
                            nc.vector.reduce_sum(out=sums[:],
                                                 in_=_ap(att[:], [[PADS, 8], [1, S]]),
                                                 axis=AX.X)
                            inv = p2s.tile([100, 8], F32)
                            nc.vector.reciprocal(out=inv[:], in_=sums[:])
                            attn = p2s.tile([100, 8 * PADS], BF16)
                            nc.vector.tensor_tensor(
                                out=_ap(attn[:], [[PADS, 8], [1, S]]),
                                in0=_ap(att[:], [[PADS, 8], [1, S]]),
                                in1=_ap(inv[:], [[1, 8], [0, S]]),
                                op=ALU.mult)
                            for pr in range(4):
                                pt = pst.tile([128, 100], BF16)
                                nc.tensor.transpose(
                                    pt[:], attn[:, pr * 128:(pr + 1) * 128],
                                    ident[0:100, 0:100])
                                nc.vector.tensor_copy(
                                    out=attnT[:, h * 400 + pr * 100:
                                              h * 400 + pr * 100 + 100],
                                    in_=pt[:])
                        for lb in range(8):
                            b = bg * 8 + lb
                            po = pso.tile([96, 1024], F32)
                            for h in range(H):
                                vsl = Vp[(lb % 2) * 64:(lb % 2) * 64 + S,
                                         (b // 2) * 768 + h * 96:
                                         (b // 2) * 768 + h * 96 + 96]
                                nc.tensor.matmul(
                                    po[:, h * 128:h * 128 + 100], vsl,
                                    attnT[(lb % 2) * 64:(lb % 2) * 64 + S,
                                          h * 400 + (lb // 2) * 100:
                                          h * 400 + (lb // 2) * 100 + 100],
                                    start=True, stop=True)
                            dst = _ap(oT[:, b * 100:], [[R, 8], [1, 100]])
                            nc.vector.tensor_copy(
                                out=dst, in_=_ap(po[:], [[128, 8], [1, 100]]))

        # ---- P3: attn_out + LN2 + FFN + LN3 -> h3T ----
        with pool("p3w") as p3w, pool("p3t") as p3t, \
             pool("p3f") as p3f, pool("p3s", bufs=2) as p3s, \
             pool("p3ps", bufs=4, space="PSUM") as p3ps, \
             pool("p3st", space="PSUM") as p3st, \
             pool("p3ab", space="PSUM") as p3ab:
            wao = p3w.tile([96, 8 * 768], BF16)
            nc.sync.dma_start(out=wao, in_=d["wao"])
            w1 = p3w.tile([128, 6 * 2048], BF16)
            nc.sync.dma_start(out=w1, in_=d["w1"])
            w2 = p3w.tile([128, 16 * 768], BF16)
            nc.sync.dma_start(out=w2, in_=d["w2"])

            def layer_norm_T(xin, gname, bname, yout):
                sq = p3f.tile([128, 6 * RC], BF16)
                nc.scalar.square(out=sq[:], in_=xin[:])
                s1 = p3st.tile([1, RC], F32)
                s2 = p3st.tile([1, RC], F32)
                for k in range(6):
                    nc.tensor.matmul(s1[:], ones_col[:],
                                     xin[:, k * RC:(k + 1) * RC],
                                     start=(k == 0), stop=(k == 5))
                for k in range(6):
                    nc.tensor.matmul(s2[:], ones_col[:],
                                     sq[:, k * RC:(k + 1) * RC],
                                     start=(k == 0), stop=(k == 5))
                mean = p3f.tile([1, RC], F32)
                nc.vector.tensor_scalar_mul(out=mean[:], in0=s1[:],
                                            scalar1=1.0 / D)
                var = p3f.tile([1, RC], F32)
                nc.vector.tensor_scalar_mul(out=var[:], in0=s2[:],
                                            scalar1=1.0 / D)
                msq = p3f.tile([1, RC], F32)
                nc.vector.tensor_tensor(out=msq[:], in0=mean[:], in1=mean[:],
                                        op=ALU.mult)
                nc.vector.tensor_tensor(out=var[:], in0=var[:], in1=msq[:],
                                        op=ALU.subtract)
                sd = p3f.tile([1, RC], F32)
                nc.scalar.activation(out=sd[:], in_=var[:], func=AF.Sqrt,
                                     bias=eps_t[:], scale=1.0)
                rstd = p3f.tile([1, RC], F32)
                nc.vector.reciprocal(out=rstd[:], in_=sd[:])
                nmr = p3f.tile([1, RC], F32)
                nc.vector.tensor_tensor(out=nmr[:], in0=mean[:], in1=rstd[:],
                                        op=ALU.mult)
                rstd_b = p3f.tile([1, RC], BF16)
                nc.vector.tensor_copy(out=rstd_b[:], in_=rstd[:])
                nmr_b = p3f.tile([1, RC], BF16)
                nc.vector.tensor_scalar_mul(out=nmr_b[:], in0=nmr[:], scalar1=-1.0)
                pa = p3ab.tile([128, RC], F32)
                nc.tensor.matmul(pa[:], ones_row[:], rstd_b[:],
                                 start=True, stop=True)
                pb = p3ab.tile([128, RC], F32)
                nc.tensor.matmul(pb[:], ones_row[:], nmr_b[:],
                                 start=True, stop=True)
                gv, bv = small[gname], small[bname]
                for k in range(6):
                    u = p3s.tile([128, RC], F32)
                    nc.vector.tensor_tensor(out=u[:],
                                            in0=xin[:, k * RC:(k + 1) * RC],
                                            in1=pa[:], op=ALU.mult)
                    if ln_triv:
                        nc.vector.tensor_tensor(out=yout(k), in0=u[:],
                                                in1=pb[:], op=ALU.add)
                    else:
                        nc.vector.tensor_tensor(out=u[:], in0=u[:], in1=pb[:],
                                                op=ALU.add)
                        nc.vector.tensor_scalar(out=yout(k), in0=u[:],
                                                scalar1=gv[:, k:k + 1],
                                                scalar2=bv[:, k:k + 1],
                                                op0=ALU.mult, op1=ALU.add)

            for c in range(NCHUNK):
                t2 = p3t.tile([128, 6 * RC], BF16)
                for m in range(6):
                    ps = p3ps.tile([128, RC], F32)
                    for kh in range(H):
                        nc.tensor.matmul(
                            ps[:],
                            wao[:, kh * 768 + m * 128:kh * 768 + m * 128 + 128],
                            oT[:, kh * R + c * RC:kh * R + (c + 1) * RC],
                            start=(kh == 0), stop=(kh == 7))
                    ta = p3s.tile([128, RC], BF16)
                    nc.scalar.activation(out=ta[:], in_=ps[:], func=AF.Identity,
                                         bias=small["bao"][:, m:m + 1], scale=1.0)
                    tg = small["tgt0"][:, m * 100:(m + 1) * 100]
                    nc.vector.tensor_tensor(out=t2[:, m * RC:(m + 1) * RC],
                                            in0=ta[:],
                                            in1=_ap(tg, [[0, 4], [1, 100]]),
                                            op=ALU.add)
                y2 = p3t.tile([128, 6 * RC], BF16)
                if ffn_triv:
                    # b1=b2=0 and trivial LN gains: relu is positive-
                    # homogeneous and LN3 is row-scale invariant, so LN2's
                    # rstd can be dropped entirely; center by mean only.
                    s1 = p3st.tile([1, RC], F32)
                    for k in range(6):
                        nc.tensor.matmul(s1[:], ones_col[:],
                                         t2[:, k * RC:(k + 1) * RC],
                                         start=(k == 0), stop=(k == 5))
                    nmean_b = p3f.tile([1, RC], BF16)
                    nc.vector.tensor_scalar_mul(out=nmean_b[:], in0=s1[:],
                                                scalar1=-1.0 / D)
                    pb = p3ab.tile([128, RC], F32)
                    nc.tensor.matmul(pb[:], ones_row[:], nmean_b[:],
                                     start=True, stop=True)
                    for k in range(6):
                        nc.vector.tensor_tensor(
                            out=y2[:, k * RC:(k + 1) * RC],
                            in0=t2[:, k * RC:(k + 1) * RC],
                            in1=pb[:], op=ALU.add)
                else:
                    layer_norm_T(t2, "ln2g", "ln2b",
                                 lambda k: y2[:, k * RC:(k + 1) * RC])
                ff1 = p3f.tile([128, 16 * RC], BF16)
                for mf in range(16):
                    ps = p3ps.tile([128, RC], F32)
                    for k in range(6):
                        nc.tensor.matmul(
                            ps[:],
                            w1[:, k * 2048 + mf * 128:k * 2048 + mf * 128 + 128],
                            y2[:, k * RC:(k + 1) * RC],
                            start=(k == 0), stop=(k == 5))
                    nc.scalar.activation(out=ff1[:, mf * RC:(mf + 1) * RC],
                                         in_=ps[:], func=AF.Relu,
                                         bias=small["b1"][:, mf:mf + 1],
                                         scale=1.0)
                t3 = p3t.tile([128, 6 * RC], BF16)
                for m in range(6):
                    ps = p3ps.tile([128, RC], F32)
                    for k in range(16):
                        nc.tensor.matmul(
                            ps[:],
                            w2[:, k * 768 + m * 128:k * 768 + m * 128 + 128],
                            ff1[:, k * RC:(k + 1) * RC],
                            start=(k == 0), stop=(k == 15))
                    tb = p3s.tile([128, RC], BF16)
                    nc.scalar.activation(out=tb[:], in_=ps[:], func=AF.Identity,
                                         bias=small["b2"][:, m:m + 1], scale=1.0)
                    nc.vector.tensor_tensor(out=t3[:, m * RC:(m + 1) * RC],
                                            in0=tb[:],
                                            in1=y2[:, m * RC:(m + 1) * RC],
                                            op=ALU.add)
                if ffn_triv:
                    # defer rstd3 to the GroupFC evacuation: center t3 only,
                    # stash rstd per row (scale commutes with h3 @ dup_g,
                    # dup_bias==0 guaranteed by the skip_dupb gate below)
                    sq = p3f.tile([128, 6 * RC], BF16)
                    nc.scalar.square(out=sq[:], in_=t3[:])
                    s1 = p3st.tile([1, RC], F32)
                    s2 = p3st.tile([1, RC], F32)
                    for k in range(6):
                        nc.tensor.matmul(s1[:], ones_col[:],
                                         t3[:, k * RC:(k + 1) * RC],
                                         start=(k == 0), stop=(k == 5))
                    for k in range(6):
                        nc.tensor.matmul(s2[:], ones_col[:],
                                         sq[:, k * RC:(k + 1) * RC],
                                         start=(k == 0), stop=(k == 5))
                    mean = p3f.tile([1, RC], F32)
                    nc.vector.tensor_scalar_mul(out=mean[:], in0=s1[:],
                                                scalar1=1.0 / D)
                    var = p3f.tile([1, RC], F32)
                    nc.vector.tensor_scalar_mul(out=var[:], in0=s2[:],
                                                scalar1=1.0 / D)
                    msq = p3f.tile([1, RC], F32)
                    nc.vector.tensor_tensor(out=msq[:], in0=mean[:],
                                            in1=mean[:], op=ALU.mult)
                    nc.vector.tensor_tensor(out=var[:], in0=var[:], in1=msq[:],
                                            op=ALU.subtract)
                    sd = p3f.tile([1, RC], F32)
                    nc.scalar.activation(out=sd[:], in_=var[:], func=AF.Sqrt,
                                         bias=eps_t[:], scale=1.0)
                    rstd = p3f.tile([1, RC], F32)
                    nc.vector.reciprocal(out=rstd[:], in_=sd[:])
                    nc.vector.tensor_copy(
                        out=rstd_all[:, c * RC:(c + 1) * RC], in_=rstd[:])
                    nmean_b = p3f.tile([1, RC], BF16)
                    nc.vector.tensor_scalar_mul(out=nmean_b[:], in0=s1[:],
                                                scalar1=-1.0 / D)
                    pb = p3ab.tile([128, RC], F32)
                    nc.tensor.matmul(pb[:], ones_row[:], nmean_b[:],
                                     start=True, stop=True)
                    for k in range(6):
                        nc.vector.tensor_tensor(
                            out=h3T[:, k * R + c * RC:k * R + (c + 1) * RC],
                            in0=t3[:, k * RC:(k + 1) * RC],
                            in1=pb[:], op=ALU.add)
                else:
                    layer_norm_T(t3, "ln3g", "ln3b",
                                 lambda k: h3T[:, k * R + c * RC:k * R + (c + 1) * RC])

        # ---- P4: GroupFC -> logitsT ----
        with pool("p4d", bufs=16) as p4d, pool("p4o") as p4o, \
             pool("p4rs_sb", bufs=2) as p4rs_sb, \
             pool("p4ps", bufs=2, space="PSUM") as p4ps, \
             pool("p4rs", bufs=2, space="PSUM") as p4rs:
            logitsT = p4o.tile([96, G * BL], F32)
            dupb = p4o.tile(list(d["dupb"].shape), BF16)
            nc.sync.dma_start(out=dupb, in_=d["dupb"])
            for g0 in range(0, G, 16):
                ng = min(16, G - g0)
                ps = p4ps.tile([96, 16 * BL], F32)
                for gi in range(ng):
                    g = g0 + gi
                    dup = p4d.tile([128, 6 * 96], BF16)
                    nc.sync.dma_start(out=dup, in_=d["dup"][g])
                    if not skip_dupb:
                        nc.tensor.matmul(ps[:, gi * BL:(gi + 1) * BL],
                                         dupb[:, g * 96:(g + 1) * 96],
                                         ones32[:], start=True, stop=False)
                    for k in range(6):
                        hsl = _ap(h3T[:, k * R + g:], [[100, BL]])
                        nc.tensor.matmul(ps[:, gi * BL:(gi + 1) * BL],
                                         dup[:, k * 96:(k + 1) * 96],
                                         hsl, start=(skip_dupb and k == 0),
                                         stop=(k == 5))
                if ffn_triv:
                    rs_ps = p4rs.tile([96, 16 * BL], F32)
                    rsl = rstd_all[:, g0:]
                    nc.tensor.matmul(
                        rs_ps[:, 0:ng * BL], ones_row[:, 0:96],
                        _ap(rsl, [[1, ng], [100, BL]]),
                        start=True, stop=True)
                    rs_sb = p4rs_sb.tile([96, 16 * BL], BF16)
                    nc.scalar.copy(out=rs_sb[:, 0:ng * BL],
                                   in_=rs_ps[:, 0:ng * BL])
                    nc.vector.tensor_tensor(
                        out=logitsT[:, g0 * BL:(g0 + ng) * BL],
                        in0=ps[:, 0:ng * BL], in1=rs_sb[:, 0:ng * BL],
                        op=ALU.mult)
                else:
                    nc.vector.tensor_copy(out=logitsT[:, g0 * BL:(g0 + ng) * BL],
                                          in_=ps[:, 0:ng * BL])
            nc.sync.dma_start(out=out_d, in_=logitsT[:])


_CACHE = {}


def kernel(**inputs):
    f32 = lambda k: np.asarray(inputs[k], np.float32)
    x = f32("x")
    w_qkv, b_qkv = f32("w_qkv"), f32("b_qkv")
    w_attn_out, b_attn_out = f32("w_attn_out"), f32("b_attn_out")

    # host constant folding for the batch-independent query path
    t = 2.0 * f32("query_embed")
    mu = t.mean(-1, keepdims=True)
    va = ((t - mu) ** 2).mean(-1, keepdims=True)
    tgt0 = (t - mu) / np.sqrt(va + EPS) * f32("ln1_g") + f32("ln1_b")
    q = (tgt0 @ w_qkv[:, :D] + b_qkv[:D]) / np.sqrt(float(HD))
    bk = b_qkv[D:2 * D]
    qbk = np.stack([q[:, h * HD:(h + 1) * HD] @ bk[h * HD:(h + 1) * HD]
                    for h in range(H)], axis=1)
    bv = b_qkv[2 * D:]
    bao_eff = b_attn_out + bv @ w_attn_out   # softmax rows sum to 1

    col6 = lambda a: np.ascontiguousarray(a.reshape(6, 128).T)
    feed = {
        "wemb": _bf(f32("w_embed").reshape(16, 128, 768).transpose(1, 0, 2)
                    .reshape(128, -1)),
        "be": col6(f32("b_embed")),
        "wk": _bf(w_qkv[:, D:2 * D].reshape(6, 128, 768).transpose(1, 0, 2)
                  .reshape(128, -1)),
        "wv": _bf(w_qkv[:, 2 * D:].reshape(6, 128, 768).transpose(1, 0, 2)
                  .reshape(128, -1)),
        "wao": _bf(w_attn_out.reshape(8, 96, 768).transpose(1, 0, 2)
                   .reshape(96, -1)),
        "bao": col6(bao_eff),
        "w1": _bf(f32("w1").reshape(6, 128, 2048).transpose(1, 0, 2)
                  .reshape(128, -1)),
        "b1": np.ascontiguousarray(f32("b1").reshape(16, 128).T),
        "w2": _bf(f32("w2").reshape(16, 128, 768).transpose(1, 0, 2)
                  .reshape(128, -1)),
        "b2": col6(f32("b2")),
        "qT": _bf(q.T.reshape(8, 96, 100).transpose(1, 0, 2).reshape(96, -1)),
        "qbk": np.ascontiguousarray(qbk.astype(np.float32)),
        "tgt0": _bf(tgt0.T.reshape(6, 128, 100).transpose(1, 0, 2)
                    .reshape(128, -1)),
        "ln2g": col6(f32("ln2_g")), "ln2b": col6(f32("ln2_b")),
        "ln3g": col6(f32("ln3_g")), "ln3b": col6(f32("ln3_b")),
        "dup": _bf(f32("dup_pool").reshape(G, 6, 128, 96).transpose(0, 2, 1, 3)
                   .reshape(G, 128, 6 * 96)),
        "dupb": _bf(f32("dup_bias").reshape(1, -1)),
    }

    skip_dupb = bool(np.all(f32("dup_bias") == 0.0))
    ln_triv = bool(np.all(f32("ln2_g") == 1.0) and np.all(f32("ln2_b") == 0.0)
                   and np.all(f32("ln3_g") == 1.0) and np.all(f32("ln3_b") == 0.0))
    ffn_triv = bool(ln_triv and np.all(f32("b1") == 0.0)
                    and np.all(f32("b2") == 0.0))
    key = ("nc", skip_dupb, ln_triv, ffn_triv)
    if key not in _CACHE:
        _CACHE[key] = build_program(skip_dupb, ln_triv, ffn_triv)
    nc = _CACHE[key]
    _CACHE["nc"] = nc

    # xr[core] axes: [c, col, k, p]; device wants [c, p, k, col]
    xr = x.reshape(NCORES, XCH, XCOLS, 16, 128)
    in_maps = []
    for core in range(NCORES):
        xT = xr[core].transpose(0, 3, 2, 1).reshape(XCH, 128, 16 * XCOLS)
        in_maps.append({**feed, "xT": _bf(xT)})

    _CACHE["in_maps"] = in_maps
    res = run_bass_kernel_spmd(nc, in_maps, list(range(NCORES)))
    outs = []
    for core in range(NCORES):
        lt = np.asarray(res.results[core]["logitsT"], np.float32)
        outs.append(lt.reshape(96, G, BL).transpose(2, 1, 0).reshape(BL, G * DF))
    return np.concatenate(outs, axis=0).astype(np.float32)


# revision 15
# speedup vs baseline: 1.2890x; 1.0105x over previous
"""Trainium2 Bass kernel for nn_MLDecoder (moe_routing).

Data-parallel over batch across 8 NeuronCores (32 batch rows/core, head params
replicated). Activations stay feature-major ("transposed"): C^T = W^T A^T via
matmul(out=C^T, lhsT=W(natural), rhs=A^T). Rows r = b*100+g (b-major). The
batch-independent query path (tgt0, q) is constant-folded on the host. All
matmuls bf16 with fp32 PSUM; LN stats via ones-matmuls; softmax without
max-subtraction (scores are O(1) for this head).
"""
import sys
sys.path.insert(0, "/opt/trn_rl_repo")

import numpy as np
import ml_dtypes

import concourse.bass as bass
from concourse import bacc
import concourse.tile as tile
import concourse.mybir as mybir
from concourse.bass import AP
from concourse.bass_utils import run_bass_kernel_spmd
from concourse.masks import make_identity

F32 = mybir.dt.float32
BF16 = mybir.dt.bfloat16
BF = ml_dtypes.bfloat16
AF = mybir.ActivationFunctionType
ALU = mybir.AluOpType
AX = mybir.AxisListType

B, S, C0 = 256, 49, 2048
D, F = 768, 2048
G, DF = 100, 96
H, HD = 8, 96
EPS = 1e-5
NCORES = 8
BL = B // NCORES          # 32 batch rows per core
R = BL * G                # 3200 rows (b,g) per core
RC = 400                  # row chunk = 4 b
NCHUNK = R // RC
XCH = 4                   # x col chunks (8 b each)
XCOLS = (BL // XCH) * S   # 392
PADS = 64                 # padded spatial stride
MCOLS = BL * PADS         # 2048 padded mem cols


def _bf(a):
    return np.ascontiguousarray(a.astype(BF))


def _ap(base, free_dims):
    """Replace the free dims of a (sliced) AP, keeping its partition dim."""
    return AP(tensor=base.tensor, offset=base.offset,
              ap=[base.ap[0]] + [list(fd) for fd in free_dims])


def build_program(skip_dupb=False, ln_triv=False, ffn_triv=False):
    nc = bacc.Bacc("TRN2", target_bir_lowering=False, debug=False,
                   num_devices=NCORES)
    d = {}

    def din(name, shape, dt):
        d[name] = nc.dram_tensor(name, list(shape), dt, kind="ExternalInput").ap()

    din("xT", (XCH, 128, 16 * XCOLS), BF16)
    din("wemb", (128, 16 * 768), BF16)
    din("be", (128, 6), F32)
    din("wk", (128, 6 * 768), BF16)
    din("wv", (128, 6 * 768), BF16)
    din("wao", (96, 8 * 768), BF16)
    din("bao", (128, 6), F32)
    din("w1", (128, 6 * 2048), BF16)
    din("b1", (128, 16), F32)
    din("w2", (128, 16 * 768), BF16)
    din("b2", (128, 6), F32)
    din("qT", (96, 8 * 100), BF16)
    din("qbk", (100, 8), F32)
    din("tgt0", (128, 6 * 100), BF16)
    din("ln2g", (128, 6), F32)
    din("ln2b", (128, 6), F32)
    din("ln3g", (128, 6), F32)
    din("ln3b", (128, 6), F32)
    din("dup", (100, 128, 6 * 96), BF16)
    din("dupb", (1, G * 96), BF16)
    out_d = nc.dram_tensor("logitsT", [96, G * BL], F32,
                           kind="ExternalOutput").ap()

    with tile.TileContext(nc) as tc:
        build_kernel(tc, d, out_d, skip_dupb, ln_triv, ffn_triv)
    nc.compile()
    return nc


def build_kernel(tc, d, out_d, skip_dupb=False, ln_triv=False, ffn_triv=False):
    nc = tc.nc

    def pool(name, bufs=1, space="SBUF"):
        return tc.tile_pool(name=name, bufs=bufs, space=space)

    with pool("resident") as res, pool("h3pool") as h3p, pool("oTpool") as oTp:
        ident = res.tile([128, 128], BF16)
        make_identity(nc, ident[:])
        ones_col = res.tile([128, 1], BF16)
        nc.vector.memset(ones_col[:], 1.0)
        ones_row = res.tile([1, 128], BF16)
        nc.vector.memset(ones_row[:], 1.0)
        ones32 = res.tile([1, BL], BF16)
        nc.vector.memset(ones32[:], 1.0)
        eps_t = res.tile([1, 1], F32)
        nc.vector.memset(eps_t[:], EPS)

        small = {}
        for name in ["be", "bao", "b1", "b2", "qT", "qbk", "tgt0",
                     "ln2g", "ln2b", "ln3g", "ln3b"]:
            t = res.tile(list(d[name].shape), d[name].dtype, tag=name)
            nc.gpsimd.dma_start(out=t, in_=d[name])
            small[name] = t

        h3T = h3p.tile([128, 6 * R], BF16)
        rstd_all = h3p.tile([1, R], BF16)
        oT = oTp.tile([96, 8 * R], BF16)

        with pool("memTpool") as memp:
            memT = memp.tile([128, 6 * MCOLS], BF16)

            # ---- P0: mem^T = relu(We^T x^T + be), written b-padded ----
            with pool("p0w") as p0w, pool("p0x", bufs=2) as p0x, \
                 pool("p0ps", bufs=3, space="PSUM") as p0ps:
                wemb = p0w.tile([128, 16 * 768], BF16)
                nc.sync.dma_start(out=wemb, in_=d["wemb"])
                for c in range(XCH):
                    xt = p0x.tile([128, 16 * XCOLS], BF16)
                    nc.sync.dma_start(out=xt, in_=d["xT"][c])
                    for m in range(6):
                        ps = p0ps.tile([128, XCOLS], F32)
                        for k in range(16):
                            nc.tensor.matmul(
                                ps[:],
                                wemb[:, k * 768 + m * 128:k * 768 + m * 128 + 128],
                                xt[:, k * XCOLS:(k + 1) * XCOLS],
                                start=(k == 0), stop=(k == 15))
                        dst = _ap(memT[:, m * MCOLS + c * 8 * PADS:],
                                  [[PADS, 8], [1, S]])
                        src = _ap(ps[:], [[S, 8], [1, S]])
                        nc.scalar.activation(out=dst, in_=src, func=AF.Relu,
                                             bias=small["be"][:, m:m + 1],
                                             scale=1.0)

            # ---- P1: K^T (head-major, b-padded) and V (rows padded) ----
            with pool("kvpool") as kvp:
                KT = kvp.tile([96, 8 * MCOLS], BF16)
                Vp = kvp.tile([128, 16 * 768], BF16)
                with pool("p1w") as p1w, \
                     pool("p1ps", bufs=3, space="PSUM") as p1ps:
                    wk = p1w.tile([128, 6 * 768], BF16)
                    nc.sync.dma_start(out=wk, in_=d["wk"])
                    wv = p1w.tile([128, 6 * 768], BF16)
                    nc.sync.dma_start(out=wv, in_=d["wv"])
                    for h in range(H):
                        for c in range(XCH):
                            ps = p1ps.tile([96, XCOLS], F32)
                            for k in range(6):
                                rhs = _ap(memT[:, k * MCOLS + c * 8 * PADS:],
                                          [[PADS, 8], [1, S]])
                                nc.tensor.matmul(
                                    ps[:],
                                    wk[:, k * 768 + h * 96:k * 768 + h * 96 + 96],
                                    rhs, start=(k == 0), stop=(k == 5))
                            dst = _ap(KT[:, h * MCOLS + c * 8 * PADS:],
                                      [[PADS, 8], [1, S]])
                            nc.vector.tensor_copy(
                                out=dst, in_=_ap(ps[:], [[S, 8], [1, S]]))
                    for t in range(16):
                        ps = p1ps.tile([128, 768], F32)
                        for sub in range(2):
                            n0, n1 = sub * 512, min(768, (sub + 1) * 512)
                            for k in range(6):
                                nc.tensor.matmul(
                                    ps[:, n0:n1],
                                    memT[:, k * MCOLS + t * 128:
                                         k * MCOLS + t * 128 + 128],
                                    wv[:, k * 768 + n0:k * 768 + n1],
                                    start=(k == 0), stop=(k == 5))
                        nc.vector.tensor_copy(out=Vp[:, t * 768:(t + 1) * 768],
                                              in_=ps[:])

                # ---- P2: attention ----
                with pool("p2a", bufs=2) as p2a, pool("p2s", bufs=3) as p2s, \
                     pool("p2ps", bufs=2, space="PSUM") as psc, \
                     pool("p2pt", bufs=2, space="PSUM") as pst, \
                     pool("p2po", bufs=2, space="PSUM") as pso:
                    for bg in range(4):
                        attnT = p2a.tile([128, 8 * 400], BF16)
                        for h in range(H):
                            ps = psc.tile([100, 8 * S], F32)
                            rhs = _ap(KT[:, h * MCOLS + bg * 8 * PADS:],
                                      [[PADS, 8], [1, S]])
                            nc.tensor.matmul(ps[:],
                                             small["qT"][:, h * 100:(h + 1) * 100],
                                             rhs, start=True, stop=True)
                            # exp into 64-padded slots (pads hold garbage,
                            # excluded by every later access pattern)
                            att = p2s.tile([100, 8 * PADS], BF16)
                            nc.scalar.activation(out=_ap(att[:], [[PADS, 8], [1, S]]),
                                                 in_=ps[:],
                                                 func=AF.Exp,
                                                 bias=small["qbk"][:, h:h + 1],
                                                 scale=1.0)
                            sums = p2s.tile([100, 8], F32)
                            nc.vector.reduce_sum(out=sums[:],
                                                 in_=_ap(att[:], [[PADS, 8], [1, S]]),
                                                 axis=AX.X)
                            inv = p2s.tile([100, 8], F32)
                            nc.vector.reciprocal(out=inv[:], in_=sums[:])
                            attn = p2s.tile([100, 8 * PADS], BF16)
                            nc.vector.tensor_tensor(
                                out=_ap(attn[:], [[PADS, 8], [1, S]]),
                                in0=_ap(att[:], [[PADS, 8], [1, S]]),
                                in1=_ap(inv[:], [[1, 8], [0, S]]),
                                op=ALU.mult)
                            for pr in range(4):
                                pt = pst.tile([128, 100], BF16)
                                nc.tensor.transpose(
                                    pt[:], attn[:, pr * 128:(pr + 1) * 128],
                                    ident[0:100, 0:100])
                                nc.vector.tensor_copy(
                                    out=attnT[:, h * 400 + pr * 100:
                                              h * 400 + pr * 100 + 100],
                                    in_=pt[:])
                        for lb in range(8):
                            b = bg * 8 + lb
                            po = pso.tile([96, 1024], F32)
                            for h in range(H):
                                vsl = Vp[(lb % 2) * 64:(lb % 2) * 64 + S,
                                         (b // 2) * 768 + h * 96:
                                         (b // 2) * 768 + h * 96 + 96]
                                nc.tensor.matmul(
                                    po[:, h * 128:h * 128 + 100], vsl,
                                    attnT[(lb % 2) * 64:(lb % 2) * 64 + S,
                                          h * 400 + (lb // 2) * 100:
                                          h * 400 + (lb // 2) * 100 + 100],
                                    start=True, stop=True)
                            dst = _ap(oT[:, b * 100:], [[R, 8], [1, 100]])
                            nc.vector.tensor_copy(
                                out=dst, in_=_ap(po[:], [[128, 8], [1, 100]]))

        # ---- P3: attn_out + LN2 + FFN + LN3 -> h3T ----
        with pool("p3w") as p3w, pool("p3t") as p3t, \
             pool("p3f") as p3f, pool("p3s", bufs=2) as p3s, \
             pool("p3ps", bufs=4, space="PSUM") as p3ps, \
             pool("p3st", space="PSUM") as p3st, \
             pool("p3ab", space="PSUM") as p3ab:
            wao = p3w.tile([96, 8 * 768], BF16)
            nc.sync.dma_start(out=wao, in_=d["wao"])
            w1 = p3w.tile([128, 6 * 2048], BF16)
            nc.sync.dma_start(out=w1, in_=d["w1"])
            w2 = p3w.tile([128, 16 * 768], BF16)
            nc.sync.dma_start(out=w2, in_=d["w2"])

            def layer_norm_T(xin, gname, bname, yout):
                sq = p3f.tile([128, 6 * RC], BF16)
                nc.scalar.square(out=sq[:], in_=xin[:])
                s1 = p3st.tile([1, RC], F32)
                s2 = p3st.tile([1, RC], F32)
                for k in range(6):
                    nc.tensor.matmul(s1[:], ones_col[:],
                                     xin[:, k * RC:(k + 1) * RC],
                                     start=(k == 0), stop=(k == 5))
                for k in range(6):
                    nc.tensor.matmul(s2[:], ones_col[:],
                                     sq[:, k * RC:(k + 1) * RC],
                                     start=(k == 0), stop=(k == 5))
                mean = p3f.tile([1, RC], F32)
                nc.vector.tensor_scalar_mul(out=mean[:], in0=s1[:],
                                            scalar1=1.0 / D)
                var = p3f.tile([1, RC], F32)
                nc.vector.tensor_scalar_mul(out=var[:], in0=s2[:],
                                            scalar1=1.0 / D)
                msq = p3f.tile([1, RC], F32)
                nc.vector.tensor_tensor(out=msq[:], in0=mean[:], in1=mean[:],
                                        op=ALU.mult)
                nc.vector.tensor_tensor(out=var[:], in0=var[:], in1=msq[:],
                                        op=ALU.subtract)
                sd = p3f.tile([1, RC], F32)
                nc.scalar.activation(out=sd[:], in_=var[:], func=AF.Sqrt,
                                     bias=eps_t[:], scale=1.0)
                rstd = p3f.tile([1, RC], F32)
                nc.vector.reciprocal(out=rstd[:], in_=sd[:])
                nmr = p3f.tile([1, RC], F32)
                nc.vector.tensor_tensor(out=nmr[:], in0=mean[:], in1=rstd[:],
                                        op=ALU.mult)
                rstd_b = p3f.tile([1, RC], BF16)
                nc.vector.tensor_copy(out=rstd_b[:], in_=rstd[:])
                nmr_b = p3f.tile([1, RC], BF16)
                nc.vector.tensor_scalar_mul(out=nmr_b[:], in0=nmr[:], scalar1=-1.0)
                pa = p3ab.tile([128, RC], F32)
                nc.tensor.matmul(pa[:], ones_row[:], rstd_b[:],
                                 start=True, stop=True)
                pb = p3ab.tile([128, RC], F32)
                nc.tensor.matmul(pb[:], ones_row[:], nmr_b[:],
                                 start=True, stop=True)
                gv, bv = small[gname], small[bname]
                for k in range(6):
                    u = p3s.tile([128, RC], F32)
                    nc.vector.tensor_tensor(out=u[:],
                                            in0=xin[:, k * RC:(k + 1) * RC],
                                            in1=pa[:], op=ALU.mult)
                    if ln_triv:
                        nc.vector.tensor_tensor(out=yout(k), in0=u[:],
                                                in1=pb[:], op=ALU.add)
                    else:
                        nc.vector.tensor_tensor(out=u[:], in0=u[:], in1=pb[:],
                                                op=ALU.add)
                        nc.vector.tensor_scalar(out=yout(k), in0=u[:],
                                                scalar1=gv[:, k:k + 1],
                                                scalar2=bv[:, k:k + 1],
                                                op0=ALU.mult, op1=ALU.add)

            for c in range(NCHUNK):
                t2 = p3t.tile([128, 6 * RC], BF16)
                for m in range(6):
                    ps = p3ps.tile([128, RC], F32)
                    for kh in range(H):
                        nc.tensor.matmul(
                            ps[:],
                            wao[:, kh * 768 + m * 128:kh * 768 + m * 128 + 128],
                            oT[:, kh * R + c * RC:kh * R + (c + 1) * RC],
                            start=(kh == 0), stop=(kh == 7))
                    ta = p3s.tile([128, RC], BF16)
                    nc.scalar.activation(out=ta[:], in_=ps[:], func=AF.Identity,
                                         bias=small["bao"][:, m:m + 1], scale=1.0)
                    tg = small["tgt0"][:, m * 100:(m + 1) * 100]
                    nc.vector.tensor_tensor(out=t2[:, m * RC:(m + 1) * RC],
                                            in0=ta[:],
                                            in1=_ap(tg, [[0, 4], [1, 100]]),
                                            op=ALU.add)
                y2 = p3t.tile([128, 6 * RC], BF16)
                if ffn_triv:
                    # b1=b2=0 and trivial LN gains: relu is positive-
                    # homogeneous and LN3 is row-scale invariant, so LN2's
                    # rstd can be dropped entirely; center by mean only.
                    s1 = p3st.tile([1, RC], F32)
                    for k in range(6):
                        nc.tensor.matmul(s1[:], ones_col[:],
                                         t2[:, k * RC:(k + 1) * RC],
                                         start=(k == 0), stop=(k == 5))
                    nmean_b = p3f.tile([1, RC], BF16)
                    nc.vector.tensor_scalar_mul(out=nmean_b[:], in0=s1[:],
                                                scalar1=-1.0 / D)
                    pb = p3ab.tile([128, RC], F32)
                    nc.tensor.matmul(pb[:], ones_row[:], nmean_b[:],
                                     start=True, stop=True)
                    for k in range(6):
                        nc.vector.tensor_tensor(
                            out=y2[:, k * RC:(k + 1) * RC],
                            in0=t2[:, k * RC:(k + 1) * RC],
                            in1=pb[:], op=ALU.add)
                else:
                    layer_norm_T(t2, "ln2g", "ln2b",
                                 lambda k: y2[:, k * RC:(k + 1) * RC])
                ff1 = p3f.tile([128, 16 * RC], BF16)
                for mf in range(16):
                    ps = p3ps.tile([128, RC], F32)
                    for k in range(6):
                        nc.tensor.matmul(
                            ps[:],
                            w1[:, k * 2048 + mf * 128:k * 2048 + mf * 128 + 128],
                            y2[:, k * RC:(k + 1) * RC],
                            start=(k == 0), stop=(k == 5))
                    nc.scalar.activation(out=ff1[:, mf * RC:(mf + 1) * RC],
                                         in_=ps[:], func=AF.Relu,
                                         bias=small["b1"][:, mf:mf + 1],
                                         scale=1.0)
                t3 = p3t.tile([128, 6 * RC], BF16)
                for m in range(6):
                    ps = p3ps.tile([128, RC], F32)
                    for k in range(16):
                        nc.tensor.matmul(
                            ps[:],
                            w2[:, k * 768 + m * 128:k * 768 + m * 128 + 128],
                            ff1[:, k * RC:(k + 1) * RC],
                            start=(k == 0), stop=(k == 15))
                    tb = p3s.tile([128, RC], BF16)
                    nc.scalar.activation(out=tb[:], in_=ps[:], func=AF.Identity,
                                         bias=small["b2"][:, m:m + 1], scale=1.0)
                    nc.vector.tensor_tensor(out=t3[:, m * RC:(m + 1) * RC],
                                            in0=tb[:],
                                            in1=y2[:, m * RC:(m + 1) * RC],
                                            op=ALU.add)
                if ffn_triv:
                    # defer rstd3 to the GroupFC evacuation: center t3 only,
                    # stash rstd per row (scale commutes with h3 @ dup_g,
                    # dup_bias==0 guaranteed by the skip_dupb gate below)
                    sq = p3f.tile([128, 6 * RC], BF16)
                    nc.scalar.square(out=sq[:], in_=t3[:])
                    s1 = p3st.tile([1, RC], F32)
                    s2 = p3st.tile([1, RC], F32)
                    for k in range(6):
                        nc.tensor.matmul(s1[:], ones_col[:],
                                         t3[:, k * RC:(k + 1) * RC],
                                         start=(k == 0), stop=(k == 5))
                    for k in range(6):
                        nc.tensor.matmul(s2[:], ones_col[:],
                                         sq[:, k * RC:(k + 1) * RC],
                                         start=(k == 0), stop=(k == 5))
                    mean = p3f.tile([1, RC], F32)
                    nc.vector.tensor_scalar_mul(out=mean[:], in0=s1[:],
                                                scalar1=1.0 / D)
                    var = p3f.tile([1, RC], F32)
                    nc.vector.tensor_scalar_mul(out=var[:], in0=s2[:],
                                                scalar1=1.0 / D)
                    msq = p3f.tile([1, RC], F32)
                    nc.vector.tensor_tensor(out=msq[:], in0=mean[:],
                                            in1=mean[:], op=ALU.mult)
                    nc.vector.tensor_tensor(out=var[:], in0=var[:], in1=msq[:],
                                            op=ALU.subtract)
                    sd = p3f.tile([1, RC], F32)
                    nc.scalar.activation(out=sd[:], in_=var[:], func=AF.Sqrt,
                                         bias=eps_t[:], scale=1.0)
                    rstd = p3f.tile([1, RC], F32)
                    nc.vector.reciprocal(out=rstd[:], in_=sd[:])
                    nc.vector.tensor_copy(
                        out=rstd_all[:, c * RC:(c + 1) * RC], in_=rstd[:])
                    nmean_b = p3f.tile([1, RC], BF16)
                    nc.vector.tensor_scalar_mul(out=nmean_b[:], in0=s1[:],
                                                scalar1=-1.0 / D)
                    pb = p3ab.tile([128, RC], F32)
                    nc.tensor.matmul(pb[:], ones_row[:], nmean_b[:],
                                     start=True, stop=True)
                    for k in range(6):
                        nc.vector.tensor_tensor(
                            out=h3T[:, k * R + c * RC:k * R + (c + 1) * RC],
                            in0=t3[:, k * RC:(k + 1) * RC],
                            in1=pb[:], op=ALU.add)
                else:
                    layer_norm_T(t3, "ln3g", "ln3b",
                                 lambda k: h3T[:, k * R + c * RC:k * R + (c + 1) * RC])

        # ---- P4: GroupFC -> logitsT ----
        with pool("p4d", bufs=16) as p4d, pool("p4o") as p4o, \
             pool("p4rs_sb", bufs=2) as p4rs_sb, \
             pool("p4ps", bufs=2, space="PSUM") as p4ps, \
             pool("p4rs", bufs=2, space="PSUM") as p4rs:
            logitsT = p4o.tile([96, G * BL], F32)
            dupb = p4o.tile(list(d["dupb"].shape), BF16)
            nc.sync.dma_start(out=dupb, in_=d["dupb"])
            for g0 in range(0, G, 16):
                ng = min(16, G - g0)
                ps = p4ps.tile([96, 16 * BL], F32)
                for gi in range(ng):
                    g = g0 + gi
                    dup = p4d.tile([128, 6 * 96], BF16)
                    nc.sync.dma_start(out=dup, in_=d["dup"][g])
                    if not skip_dupb:
                        nc.tensor.matmul(ps[:, gi * BL:(gi + 1) * BL],
                                         dupb[:, g * 96:(g + 1) * 96],
                                         ones32[:], start=True, stop=False)
                    for k in range(6):
                        hsl = _ap(h3T[:, k * R + g:], [[100, BL]])
                        nc.tensor.matmul(ps[:, gi * BL:(gi + 1) * BL],
                                         dup[:, k * 96:(k + 1) * 96],
                                         hsl, start=(skip_dupb and k == 0),
                                         stop=(k == 5))
                if ffn_triv:
                    rs_ps = p4rs.tile([96, 16 * BL], F32)
                    rsl = rstd_all[:, g0:]
                    nc.tensor.matmul(
                        rs_ps[:, 0:ng * BL], ones_row[:, 0:96],
                        _ap(rsl, [[1, ng], [100, BL]]),
                        start=True, stop=True)
                    rs_sb = p4rs_sb.tile([96, 16 * BL], BF16)
                    nc.scalar.copy(out=rs_sb[:, 0:ng * BL],
                                   in_=rs_ps[:, 0:ng * BL])
                    nc.vector.tensor_tensor(
                        out=logitsT[:, g0 * BL:(g0 + ng) * BL],
                        in0=ps[:, 0:ng * BL], in1=rs_sb[:, 0:ng * BL],
                        op=ALU.mult)
                else:
                    nc.vector.tensor_copy(out=logitsT[:, g0 * BL:(g0 + ng) * BL],
                                          in_=ps[:, 0:ng * BL])
            nc.sync.dma_start(out=out_d, in_=logitsT[:])


_CACHE = {}


def kernel(**inputs):
    f32 = lambda k: np.asarray(inputs[k], np.float32)
    x = f32("x")
    w_qkv, b_qkv = f32("w_qkv"), f32("b_qkv")
    w_attn_out, b_attn_out = f32("w_attn_out"), f32("b_attn_out")

    # host constant folding for the batch-independent query path
    t = 2.0 * f32("query_embed")
    mu = t.mean(-1, keepdims=True)
    va = ((t - mu) ** 2).mean(-1, keepdims=True)
    tgt0 = (t - mu) / np.sqrt(va + EPS) * f32("ln1_g") + f32("ln1_b")
    q = (tgt0 @ w_qkv[:, :D] + b_qkv[:D]) / np.sqrt(float(HD))
    bk = b_qkv[D:2 * D]
    qbk = np.stack([q[:, h * HD:(h + 1) * HD] @ bk[h * HD:(h + 1) * HD]
                    for h in range(H)], axis=1)
    bv = b_qkv[2 * D:]
    bao_eff = b_attn_out + bv @ w_attn_out   # softmax rows sum to 1

    col6 = lambda a: np.ascontiguousarray(a.reshape(6, 128).T)
    feed = {
        "wemb": _bf(f32("w_embed").reshape(16, 128, 768).transpose(1, 0, 2)
                    .reshape(128, -1)),
        "be": col6(f32("b_embed")),
        "wk": _bf(w_qkv[:, D:2 * D].reshape(6, 128, 768).transpose(1, 0, 2)
                  .reshape(128, -1)),
        "wv": _bf(w_qkv[:, 2 * D:].reshape(6, 128, 768).transpose(1, 0, 2)
                  .reshape(128, -1)),
        "wao": _bf(w_attn_out.reshape(8, 96, 768).transpose(1, 0, 2)
                   .reshape(96, -1)),
        "bao": col6(bao_eff),
        "w1": _bf(f32("w1").reshape(6, 128, 2048).transpose(1, 0, 2)
                  .reshape(128, -1)),
        "b1": np.ascontiguousarray(f32("b1").reshape(16, 128).T),
        "w2": _bf(f32("w2").reshape(16, 128, 768).transpose(1, 0, 2)
                  .reshape(128, -1)),
        "b2": col6(f32("b2")),
        "qT": _bf(q.T.reshape(8, 96, 100).transpose(1, 0, 2).reshape(96, -1)),
        "qbk": np.ascontiguousarray(qbk.astype(np.float32)),
        "tgt0": _bf(tgt0.T.reshape(6, 128, 100).transpose(1, 0, 2)
                    .reshape(128, -1)),
        "ln2g": col6(f32("ln2_g")), "ln2b": col6(f32("ln2_b")),
        "ln3g": col6(f32("ln3_g")), "ln3b": col6(f32("ln3_b")),
        "dup": _bf(f32("dup_pool").reshape(G, 6, 128, 96).transpose(0, 2, 1, 3)
                   .reshape(G, 128, 6 * 96)),
        "dupb": _bf(f32("dup_bias").reshape(1, -1)),
    }

    skip_dupb = bool(np.all(f32("dup_bias") == 0.0))
    ln_triv = bool(np.all(f32("ln2_g") == 1.0) and np.all(f32("ln2_b") == 0.0)
                   and np.all(f32("ln3_g") == 1.0) and np.all(f32("ln3_b") == 0.0))
    ffn_triv = bool(ln_triv and np.all(f32("b1") == 0.0)
                    and np.all(f32("b2") == 0.0))
    key = ("nc", skip_dupb, ln_triv, ffn_triv)
    if key not in _CACHE:
        _CACHE[key] = build_program(skip_dupb, ln_triv, ffn_triv)
    nc = _CACHE[key]
    _CACHE["nc"] = nc

    # xr[core] axes: [c, col, k, p]; device wants [c, p, k, col]
    xr = x.reshape(NCORES, XCH, XCOLS, 16, 128)
    in_maps = []
    for core in range(NCORES):
        xT = xr[core].transpose(0, 3, 2, 1).reshape(XCH, 128, 16 * XCOLS)
        in_maps.append({**feed, "xT": _bf(xT)})

    _CACHE["in_maps"] = in_maps
    res = run_bass_kernel_spmd(nc, in_maps, list(range(NCORES)))
    outs = []
    for core in range(NCORES):
        lt = np.asarray(res.results[core]["logitsT"], np.float32)
        outs.append(lt.reshape(96, G, BL).transpose(2, 1, 0).reshape(BL, G * DF))
    return np.concatenate(outs, axis=0).astype(np.float32)
